# revision 1
# baseline (speedup 1.0000x reference)
"""Multi-head causal self-attention (B=2, T=2048, C=1024, H=16, D=64) on 8
Trainium2 NeuronCores.

Sharding: data-parallel over batch (2) x tensor-parallel over heads (4 groups
of 4 heads) = 8 shards, no cross-core communication. Each core computes, for
its (batch b, head-group g):
    qkvT = w_slice.T @ x[b].T  (+bias)   -> qT,kT [256,2048], v [2048,256]
    per head: scoresT = kT' q ... softmax in transposed layout (causal)
    attT (unnormalized) + denominators via a ones-column in the PV matmul
    partial output = attT_norm.T @ w_proj_rows + 0.25*b_proj  -> [2048,1024]
Host sums the 4 partial outputs per batch.

Projection / score matmuls run in float32r (fp32 truncated to ~fp22 in the
PE - full rate for moving free dim >= 256). The probability*value matmul runs
in bf16 (P quantization ~0.4%) which keeps the exp on the scalar engine at 2x
and the PV matmul off the fp32r small-N penalty.

The per-tq blocks interleave qkv projection, attention, and output projection
so every engine has work throughout instead of three serial phases.
"""

import numpy as np

import concourse.bass as bass
import concourse.mybir as mybir
import concourse.tile as tile
from concourse import bacc
from concourse.bass_utils import run_bass_kernel_spmd

f32 = mybir.dt.float32
f32r = mybir.dt.float32r
bf16 = mybir.dt.bfloat16
AF = mybir.ActivationFunctionType
ALU = mybir.AluOpType

B, T, C, H, D = 2, 2048, 1024, 16, 64
HPC = 4          # heads per core
NCORES = 8
TQ = 512         # q tile (matmul moving free dim)
NTQ = T // TQ    # 4
NKC = C // 128   # 8 contraction chunks for the qkv projection
SCALE = 1.0 / 8.0  # 1/sqrt(D)

import os

USE_BF16_PV = True
_B = lambda name, dflt: int(os.environ.get(name, dflt))
PV_DT = bf16 if USE_BF16_PV else f32r

_CACHE = {}

U32_ONE = 1065353216  # np.float32(1.0).view(np.uint32)


def memset_bits(eng, ap, bits):
    """memset an fp32r AP via its uint32 bit view (ISA has no fp32r memset)."""
    eng.memset(ap.bitcast(mybir.dt.uint32), bits)


def pv_memset(eng, ap, val):
    if USE_BF16_PV:
        eng.memset(ap, val)
    else:
        memset_bits(eng, ap, U32_ONE if val == 1.0 else 0)


def build_nc(debug_taps=False):
    nc = bacc.Bacc("TRN2", target_bir_lowering=False, debug=False)

    xt_d = nc.dram_tensor("xt", [C, T], f32r, kind="ExternalInput")
    wqkv_d = nc.dram_tensor("wqkv", [C, 768], f32r, kind="ExternalInput")
    bqk_d = nc.dram_tensor("bqk", [128, 4], f32, kind="ExternalInput")
    bv_d = nc.dram_tensor("bv", [1, 256], f32r, kind="ExternalInput")
    wproj_d = nc.dram_tensor("wproj", [256, C], f32r, kind="ExternalInput")
    bprojq_d = nc.dram_tensor("bprojq", [1, C], f32r, kind="ExternalInput")
    out_d = nc.dram_tensor("out", [T, C], f32, kind="ExternalOutput")
    if debug_taps:
        dbg_qkT = nc.dram_tensor("dbg_qkT", [128, 4, T], f32, kind="ExternalOutput")
        dbg_attT = nc.dram_tensor("dbg_attT", [128, 2, T], f32, kind="ExternalOutput")
        dbg_pt = nc.dram_tensor("dbg_pt", [4, 128, TQ], f32, kind="ExternalOutput")

    with tile.TileContext(nc) as tc:
        with (
            tc.tile_pool(name="const", bufs=1) as const,
            tc.tile_pool(name="xts", bufs=_B("XTS_B", 2)) as xts_pool,
            tc.tile_pool(name="pt", bufs=_B("PT_B", 8)) as pt_pool,
            tc.tile_pool(name="bcs", bufs=_B("BCS_B", 4)) as bcs_pool,
            tc.tile_pool(name="rec", bufs=_B("REC_B", 4)) as rec_pool,
            tc.tile_pool(name="ot", bufs=_B("OT_B", 4)) as ot_pool,
            tc.tile_pool(name="attm", bufs=_B("ATTM_B", 4)) as attm_pool,
            tc.tile_pool(name="ps_a", bufs=2, space="PSUM") as ps_a,
            tc.tile_pool(name="ps_s", bufs=2, space="PSUM") as ps_s,
            tc.tile_pool(name="ps_acc", bufs=2, space="PSUM") as ps_acc,
        ):
            # ---- resident tensors; DMAs chunked so compute starts early ----
            wqkv_sb = const.tile([128, NKC, 768], f32r, tag="wqkv")
            wqkv_r = wqkv_d.rearrange("(o p) n -> p o n", p=128)
            for kc in range(NKC):
                nc.scalar.dma_start(wqkv_sb[:, kc, :], wqkv_r[:, kc, :])
            bqk_sb = const.tile([128, 4], f32, tag="bqk")
            nc.scalar.dma_start(bqk_sb[:], bqk_d[:, :])
            bv_sb = const.tile([1, 256], f32r, tag="bv")
            nc.scalar.dma_start(bv_sb[:], bv_d[:, :])
            wproj_sb = const.tile([128, 2, C], f32r, tag="wproj")
            nc.scalar.dma_start(wproj_sb[:], wproj_d.rearrange("(o p) n -> p o n", p=128))
            bprojq_sb = const.tile([1, C], f32r, tag="bprojq")
            nc.scalar.dma_start(bprojq_sb[:], bprojq_d[:, :])

            ones_sb = const.tile([128, 128], f32r, tag="ones")
            memset_bits(nc.vector, ones_sb[:], U32_ONE)
            # onespad: col 0-63 = 0, 64-127 = 1 (for odd-head recip broadcast)
            onespad_sb = const.tile([1, 128], f32r, tag="onespad")
            memset_bits(nc.vector, onespad_sb[:, 0:64], 0)
            memset_bits(nc.vector, onespad_sb[:, 64:128], U32_ONE)

            # qkT: chunk 0,1 = qT (heads 01 / 23), chunk 2,3 = kT
            qkT_sb = const.tile([128, 4, T], f32r, tag="qkT")
            # v (PV lhsT layout), indexed [p, tt, parity, pairidx, col];
            # head h lives at [:, :, h % 2, h // 2, :]:
            #   even h: cols [V(64) | ones(1)]            (rest unused)
            #   odd h:  cols [ones(1) | zeros(63) | V(64)]
            v_sb = const.tile([128, T // 128, 2, 2, 128], PV_DT, tag="v")
            pv_memset(nc.vector, v_sb[:, :, 0, :, 64:65], 1.0)
            pv_memset(nc.vector, v_sb[:, :, 1, :, 1:64], 0.0)
            pv_memset(nc.vector, v_sb[:, :, 1, :, 0:1], 1.0)
            # attT: chunk c partitions 0-63 = head 2c, 64-127 = head 2c+1
            attT_sb = const.tile([128, 2, T], f32r, tag="attT")

            xt_r = xt_d.rearrange("(o p) t -> p o t", p=128)

            def emit_qkv(tq):
                tqs = slice(TQ * tq, TQ * (tq + 1))
                # ---- qkv projection for this t-slice ----------------------
                xts = xts_pool.tile([128, NKC, TQ], f32r, tag="xts")
                for kc in range(NKC):
                    nc.sync.dma_start(xts[:, kc, :], xt_r[:, kc, tqs])
                # q,k transposed: psum = wqkv_chunk.T @ xT
                if tq == 0:
                    # kc-major with 4 open psum groups: the first matmul only
                    # needs the first 128-row chunk of x/wqkv, hiding the
                    # initial DMA ramp
                    qk_ps = []
                    for cp in range(4):
                        pool_ = ps_a if cp < 2 else ps_acc
                        tag_ = "ps_a" if cp < 2 else "acc"
                        qkp = pool_.tile([128, TQ], f32, tag=tag_, name=f"qkp{cp}")
                        qk_ps.append(qkp)
                    for kc in range(NKC):
                        for cp in range(4):
                            nc.tensor.matmul(
                                qk_ps[cp][:],
                                lhsT=wqkv_sb[:, kc, 128 * cp : 128 * (cp + 1)],
                                rhs=xts[:, kc, :],
                                start=(kc == 0),
                                stop=(kc == NKC - 1),
                                skip_group_check=True,
                            )
                    for cp in range(4):
                        nc.vector.tensor_scalar_add(
                            qkT_sb[:, cp, tqs], qk_ps[cp][:], bqk_sb[:, cp : cp + 1]
                        )
                else:
                    for cp in range(4):
                        ps = ps_a.tile([128, TQ], f32, tag="ps_a")
                        for kc in range(NKC):
                            nc.tensor.matmul(
                                ps[:],
                                lhsT=wqkv_sb[:, kc, 128 * cp : 128 * (cp + 1)],
                                rhs=xts[:, kc, :],
                                start=(kc == 0),
                                stop=(kc == NKC - 1),
                            )
                        nc.vector.tensor_scalar_add(
                            qkT_sb[:, cp, tqs], ps[:], bqk_sb[:, cp : cp + 1]
                        )
                # v: psum = xT_chunk.T @ wv (+ bias via K=1 matmul)
                for tt in range(4 * tq, 4 * tq + 4):
                    psv = ps_a.tile([128, TQ], f32, tag="ps_a")
                    toff = 128 * tt - TQ * tq
                    for kc in range(NKC):
                        nc.tensor.matmul(
                            psv[:, 0:256],
                            lhsT=xts[:, kc, toff : toff + 128],
                            rhs=wqkv_sb[:, kc, 512:768],
                            start=(kc == 0),
                            stop=False,
                        )
                    nc.tensor.matmul(
                        psv[:, 0:256],
                        lhsT=ones_sb[0:1, 0:128],
                        rhs=bv_sb[0:1, :],
                        start=False,
                        stop=True,
                    )
                    # psv v-columns are host-ordered [h0|h2|h1|h3] so each
                    # parity lands in one strided copy
                    nc.vector.tensor_copy(
                        v_sb[:, tt, 0, :, 0:64], psv[:, 0:128]
                    )
                    nc.vector.tensor_copy(
                        v_sb[:, tt, 1, :, 64:128], psv[:, 128:256]
                    )

            def emit_att(tq):
                tqs = slice(TQ * tq, TQ * (tq + 1))
                # ---- attention for this q-slice ---------------------------
                for pair in range(2):
                    hA, hB = 2 * pair, 2 * pair + 1
                    accA = ps_acc.tile([128, TQ], f32, tag="acc")
                    accB = ps_acc.tile([128, TQ], f32, tag="acc")
                    ntk = 4 * tq + 4
                    for tk in range(ntk):
                        d = tk - 4 * tq  # >= 0 on the diagonal block
                        q0 = 128 * d if d >= 0 else 0
                        w = TQ - q0
                        ks = slice(128 * tk, 128 * (tk + 1))
                        qs = slice(TQ * tq + q0, TQ * (tq + 1))
                        sc = ps_s.tile([128, 2, TQ], f32, tag="sc")
                        # two K=64 matmuls packed on row halves of the PE,
                        # writing the two banks of one psum tile
                        nc.tensor.matmul(
                            sc[:, 0, 0:w],
                            lhsT=qkT_sb[0:64, 2 + pair, ks],
                            rhs=qkT_sb[0:64, pair, qs],
                        )
                        nc.tensor.matmul(
                            sc[:, 1, 0:w],
                            lhsT=qkT_sb[64:128, 2 + pair, ks],
                            rhs=qkT_sb[64:128, pair, qs],
                        )
                        pt = pt_pool.tile([128, 2, TQ], PV_DT, tag="pt")
                        ptA = pt[:, 0]
                        ptB = pt[:, 1]
                        nc.scalar.activation(pt[:, :, 0:w], sc[:, :, 0:w], AF.Exp, scale=SCALE)
                        if d >= 0:
                            # zero strictly-above-diagonal in the leading
                            # 128x128 block: keep where (j - p) >= 0
                            for pt_ in (ptA, ptB):
                                nc.gpsimd.affine_select(
                                    pt_[:, 0:128],
                                    pt_[:, 0:128],
                                    pattern=[[1, 128]],
                                    compare_op=ALU.is_ge,
                                    fill=0.0,
                                    base=0,
                                    channel_multiplier=-1,
                                )
                        if debug_taps and pair == 0 and tq == 0:
                            dtp = pt_pool.tile([128, TQ], f32, tag="dbgcp")
                            nc.vector.tensor_copy(dtp[:, 0:w], ptA[:, 0:w])
                            if q0 > 0:
                                nc.vector.memset(dtp[:, 0:q0], 0.0)
                            nc.sync.dma_start(dbg_pt[tk, :, :], dtp[:])
                        st = (tk == 0)
                        sp = (tk == ntk - 1)
                        # even head: out rows 0-63 att, row 64 denom
                        nc.tensor.matmul(
                            accA[0:65, q0:TQ],
                            lhsT=v_sb[:, tk, 0, pair, 0:65],
                            rhs=ptA[:, 0:w],
                            start=st,
                            stop=sp,
                            skip_group_check=True,
                        )
                        # odd head: out row 0 denom, rows 64-127 att
                        nc.tensor.matmul(
                            accB[:, q0:TQ],
                            lhsT=v_sb[:, tk, 1, pair, :],
                            rhs=ptB[:, 0:w],
                            start=st,
                            stop=sp,
                            skip_group_check=True,
                        )
                    # copy both accumulators to SBUF immediately so the PSUM
                    # banks free up for the next pair while the (longer)
                    # normalize chain runs off the SBUF copies.
                    attmA = attm_pool.tile([128, TQ], f32r, tag="attm")
                    nc.vector.tensor_copy(attmA[0:65, :], accA[0:65, :])
                    attmB = attm_pool.tile([128, TQ], f32r, tag="attm")
                    nc.vector.tensor_copy(attmB[:, :], accB[:, :])
                    # normalize head A (denominator at partition 64).
                    # reciprocal_approx_fast is broken at base_partition != 0
                    # on HW, so broadcast the raw denominator to rows 0-63
                    # first and take the reciprocal there.
                    bcA = ps_acc.tile([128, TQ], f32, tag="acc")
                    nc.tensor.matmul(
                        bcA[0:64, :],
                        lhsT=ones_sb[64:65, 0:64],
                        rhs=attmA[64:65, :],
                    )
                    bcsA = bcs_pool.tile([128, TQ], f32, tag="bcs")
                    nc.scalar.activation(bcsA[0:64, :], bcA[0:64, :], AF.Copy)
                    nc.vector.reciprocal_approx_fast(
                        out=bcsA[0:64, :], in_=bcsA[0:64, :]
                    )
                    nc.vector.tensor_mul(
                        attT_sb[0:64, pair, tqs],
                        attmA[0:64, :].bitcast(f32),
                        bcsA[0:64, :],
                    )
                    # normalize head B (denominator at partition 0)
                    recB = rec_pool.tile([128, TQ], f32, tag="rec")
                    nc.vector.reciprocal_approx_fast(
                        out=recB[0:1, :], in_=attmB[0:1, :].bitcast(f32)
                    )
                    recBr = rec_pool.tile([128, TQ], f32r, tag="recr")
                    nc.vector.tensor_copy(recBr[0:1, :], recB[0:1, :])
                    bcB = ps_acc.tile([128, TQ], f32, tag="acc")
                    nc.tensor.matmul(
                        bcB[:, :],
                        lhsT=onespad_sb[0:1, :],
                        rhs=recBr[0:1, :],
                    )
                    bcsB = bcs_pool.tile([128, TQ], f32, tag="bcs")
                    nc.scalar.activation(bcsB[64:128, :], bcB[64:128, :], AF.Copy)
                    nc.vector.tensor_mul(
                        attT_sb[64:128, pair, tqs],
                        attmB[64:128, :].bitcast(f32),
                        bcsB[64:128, :],
                    )

            def emit_proj(tq):
                tqs = slice(TQ * tq, TQ * (tq + 1))
                # ---- output projection for this t-slice -------------------
                for tt in range(4 * tq, 4 * tq + 4):
                    ts_ = slice(128 * tt, 128 * (tt + 1))
                    for nt in range(2):
                        ns = slice(512 * nt, 512 * (nt + 1))
                        pso = ps_a.tile([128, TQ], f32, tag="ps_a")
                        for hc in range(2):
                            nc.tensor.matmul(
                                pso[:],
                                lhsT=attT_sb[:, hc, ts_],
                                rhs=wproj_sb[:, hc, ns],
                                start=(hc == 0),
                                stop=False,
                            )
                        nc.tensor.matmul(
                            pso[:],
                            lhsT=ones_sb[0:1, 0:128],
                            rhs=bprojq_sb[0:1, ns],
                            start=False,
                            stop=True,
                        )
                        ot = ot_pool.tile([128, TQ], f32, tag="ot")
                        nc.vector.tensor_copy(ot[:], pso[:])
                        nc.sync.dma_start(out_d[ts_, ns], ot[:])

            # software pipeline: qkv one block ahead of attention+proj so
            # the PE always has independent work during normalize tails
            emit_qkv(0)
            for tq in range(NTQ):
                if tq + 1 < NTQ:
                    emit_qkv(tq + 1)
                emit_att(tq)
                if tq >= 1:
                    emit_proj(tq - 1)
            emit_proj(NTQ - 1)
            if debug_taps:
                nc.sync.dma_start(dbg_qkT[:, :, :], qkT_sb[:].bitcast(f32))
                nc.sync.dma_start(dbg_attT[:, :, :], attT_sb[:].bitcast(f32))

    nc.compile()
    return nc


def _shard_inputs(x, w_qkv, b_qkv, w_proj, b_proj):
    """Full inputs -> per-core input maps. Core c = (batch b=c//4, group g=c%4)."""
    in_maps = []
    xts = [np.ascontiguousarray(x[b].T) for b in range(B)]
    bprojq = np.ascontiguousarray((0.25 * b_proj).reshape(1, C).astype(np.float32))
    for core in range(NCORES):
        b, g = divmod(core, 4)
        qs = slice(256 * g, 256 * (g + 1))
        ks = slice(C + 256 * g, C + 256 * (g + 1))
        vs = slice(2 * C + 256 * g, 2 * C + 256 * (g + 1))
        wv = w_qkv[:, vs]
        wv = np.concatenate(
            [wv[:, 0:64], wv[:, 128:192], wv[:, 64:128], wv[:, 192:256]], axis=1
        )  # head order h0|h2|h1|h3 so the v copyback is two strided copies
        wqkv = np.ascontiguousarray(
            np.concatenate([w_qkv[:, qs], w_qkv[:, ks], wv], axis=1)
        )
        bqk = np.ascontiguousarray(
            np.stack(
                [
                    b_qkv[qs][0:128],
                    b_qkv[qs][128:256],
                    b_qkv[ks][0:128],
                    b_qkv[ks][128:256],
                ],
                axis=1,
            )
        )
        bvv = b_qkv[vs]
        bv = np.ascontiguousarray(
            np.concatenate(
                [bvv[0:64], bvv[128:192], bvv[64:128], bvv[192:256]]
            ).reshape(1, 256)
        )
        wproj = np.ascontiguousarray(w_proj[256 * g : 256 * (g + 1), :])
        in_maps.append(
            {
                "xt": xts[b],
                "wqkv": wqkv.astype(np.float32),
                "bqk": bqk.astype(np.float32),
                "bv": bv.astype(np.float32),
                "wproj": wproj.astype(np.float32),
                "bprojq": bprojq,
            }
        )
    return in_maps


def kernel(x, w_qkv, b_qkv, w_proj, b_proj):
    x = np.asarray(x, dtype=np.float32)
    w_qkv = np.asarray(w_qkv, dtype=np.float32)
    b_qkv = np.asarray(b_qkv, dtype=np.float32)
    w_proj = np.asarray(w_proj, dtype=np.float32)
    b_proj = np.asarray(b_proj, dtype=np.float32)

    if "nc" not in _CACHE:
        _CACHE["nc"] = build_nc()
    nc = _CACHE["nc"]

    in_maps = _shard_inputs(x, w_qkv, b_qkv, w_proj, b_proj)
    res = run_bass_kernel_spmd(nc, in_maps, list(range(NCORES)))
    out = np.empty((B, T, C), dtype=np.float32)
    for b in range(B):
        acc = res.results[4 * b]["out"].astype(np.float32)
        for g in range(1, 4):
            acc = acc + res.results[4 * b + g]["out"]
        out[b] = acc
    return out



# revision 2
# speedup vs baseline: 1.1025x; 1.1025x over previous
"""Multi-head causal self-attention (B=2, T=2048, C=1024, H=16, D=64) on 8
Trainium2 NeuronCores.

Sharding: data-parallel over batch (2) x tensor-parallel over heads (4 groups
of 4 heads) = 8 shards, no cross-core communication. Host sums the 4 partial
outputs per batch and adds the bias.

All matmuls in bf16 (1 cycle/row in the PE at any N). Per core:
  qkT = wqk.T @ xT            [4x128, T]   (chunks: q01 q23 k01 k23)
  v   = xT.T @ wv             [T, 4, 64]+ones col (rhs layout for PV)
  per (tk 128-key block, head): scT = k_blk.T @ qT -> exp -> pt [keys, queries]
  PV non-transposed: att[q, h, 0:65] += pt[:, qblk].T @ v[:, tk, h, 0:65]
     (col 64 = ones -> per-query softmax denominator on the PSUM partition)
  normalize per qblock with per-partition reciprocal (DVE), write att bf16
  attT via DMA XBAR transpose (runs on DMA engines, not PE)
  out = attT.T @ wproj  -> bf16 out DMA; host adds b_proj + bv@wproj.

v1 did PV in transposed orientation ([65, w] out, N=w cost) plus PE broadcast
matmuls and activation copies for the denominators; v2's orientation makes PV
cost N=65 per key-block (2x fewer PE cycles) and normalization nearly free.
"""

import numpy as np
import ml_dtypes

import concourse.bass as bass
import concourse.mybir as mybir
import concourse.tile as tile
from concourse import bacc
from concourse.bass_utils import run_bass_kernel_spmd

f32 = mybir.dt.float32
bf16 = mybir.dt.bfloat16
AF = mybir.ActivationFunctionType
ALU = mybir.AluOpType

B, T, C, H, D = 2, 2048, 1024, 16, 64
HPC = 4          # heads per core
NCORES = 8
TQ = 512         # query tile of the attention outer loop
NTQ = T // TQ    # 4
NKC = C // 128   # 8 contraction chunks for the qkv projection
NTT = T // 128   # 16 query 128-blocks
SCALE = 1.0 / 8.0  # 1/sqrt(D)

_CACHE = {}


def build_nc():
    nc = bacc.Bacc("TRN2", target_bir_lowering=False, debug=False)

    xt_d = nc.dram_tensor("xt", [C, T], bf16, kind="ExternalInput")
    wqkv_d = nc.dram_tensor("wqkv", [C, 768], bf16, kind="ExternalInput")
    bqk_d = nc.dram_tensor("bqk", [128, 4], f32, kind="ExternalInput")
    wproj_d = nc.dram_tensor("wproj", [256, C], bf16, kind="ExternalInput")
    out_d = nc.dram_tensor("out", [T, C], bf16, kind="ExternalOutput")

    with tile.TileContext(nc) as tc:
        with (
            tc.tile_pool(name="const", bufs=1) as const,
            tc.tile_pool(name="xts", bufs=3) as xts_pool,
            tc.tile_pool(name="pt", bufs=16) as pt_pool,
            tc.tile_pool(name="atts", bufs=8) as atts_pool,
            tc.tile_pool(name="rec", bufs=8) as rec_pool,
            tc.tile_pool(name="ot", bufs=8) as ot_pool,
            tc.tile_pool(name="ps_mm", bufs=2, space="PSUM") as ps_mm,
            tc.tile_pool(name="ps_sc", bufs=2, space="PSUM") as ps_sc,
            tc.tile_pool(name="ps_att", bufs=4, space="PSUM") as ps_att,
        ):
            # ---- resident tensors; DMAs chunked so compute starts early ----
            wqkv_sb = const.tile([128, NKC, 768], bf16, tag="wqkv")
            wqkv_r = wqkv_d.rearrange("(o p) n -> p o n", p=128)
            # chunk 0 alone so the first matmul can start early; the rest in
            # one DMA (each DMA holds the single-slot HWDGE ~.6us regardless
            # of size, so fewer DMAs = less serialization)
            nc.scalar.dma_start(wqkv_sb[:, 0, :], wqkv_r[:, 0, :])
            nc.scalar.dma_start(wqkv_sb[:, 1:NKC, :], wqkv_r[:, 1:NKC, :])
            bqk_sb = const.tile([128, 4], f32, tag="bqk")
            nc.scalar.dma_start(bqk_sb[:], bqk_d[:, :])
            wproj_sb = const.tile([128, 2, C], bf16, tag="wproj")
            nc.scalar.dma_start(wproj_sb[:], wproj_d.rearrange("(o p) n -> p o n", p=128))

            # qkT chunks: 0 = qT heads01, 1 = qT heads23, 2 = kT h01, 3 = kT h23
            qkT_sb = const.tile([128, 4, T], bf16, tag="qkT")
            # v in PV-rhs layout: [key mod 128, key block, head, 64 vdims + one]
            v_sb = const.tile([128, NTT, HPC, 65], bf16, tag="v")
            nc.vector.memset(v_sb[:, :, :, 64:65], 1.0)
            # attT: chunk hc: partitions = head-dims of heads (2hc, 2hc+1)
            attT_sb = const.tile([128, 2, T], bf16, tag="attT")
            # identity for PE-transpose of the normalized attention
            ident_sb = const.tile([128, 128], bf16, tag="ident")
            nc.vector.memset(ident_sb[:], 1.0)
            nc.gpsimd.affine_select(
                ident_sb[:],
                ident_sb[:],
                pattern=[[1, 128]],
                compare_op=ALU.is_equal,
                fill=0.0,
                base=0,
                channel_multiplier=-1,
            )

            xt_r = xt_d.rearrange("(o p) t -> p o t", p=128)

            def emit_qkv(tq):
                tqs = slice(TQ * tq, TQ * (tq + 1))
                xts = xts_pool.tile([128, NKC, TQ], bf16, tag="xts")
                nc.sync.dma_start(xts[:, 0, :], xt_r[:, 0, tqs])
                nc.sync.dma_start(xts[:, 1:NKC, :], xt_r[:, 1:NKC, tqs])
                # q,k transposed: psum = wqkv_chunk.T @ xT
                if tq == 0:
                    # kc-major over pairs of open psum groups to hide DMA ramp
                    for cpp in range(2):
                        ps_pair = [
                            ps_mm.tile([128, TQ], f32, tag="mm", name=f"qk{cpp}{i}")
                            for i in range(2)
                        ]
                        for kc in range(NKC):
                            for i in range(2):
                                cp = 2 * cpp + i
                                nc.tensor.matmul(
                                    ps_pair[i][:],
                                    lhsT=wqkv_sb[:, kc, 128 * cp : 128 * (cp + 1)],
                                    rhs=xts[:, kc, :],
                                    start=(kc == 0),
                                    stop=(kc == NKC - 1),
                                    skip_group_check=True,
                                )
                        for i in range(2):
                            cp = 2 * cpp + i
                            nc.vector.tensor_scalar_add(
                                qkT_sb[:, cp, tqs], ps_pair[i][:], bqk_sb[:, cp : cp + 1]
                            )
                else:
                    for cp in range(4):
                        ps = ps_mm.tile([128, TQ], f32, tag="mm")
                        for kc in range(NKC):
                            nc.tensor.matmul(
                                ps[:],
                                lhsT=wqkv_sb[:, kc, 128 * cp : 128 * (cp + 1)],
                                rhs=xts[:, kc, :],
                                start=(kc == 0),
                                stop=(kc == NKC - 1),
                            )
                        nc.vector.tensor_scalar_add(
                            qkT_sb[:, cp, tqs], ps[:], bqk_sb[:, cp : cp + 1]
                        )
                # v: psum = xT_chunk.T @ wv  (no bias: folded into host output)
                for tt in range(4 * tq, 4 * tq + 4):
                    psv = ps_mm.tile([128, TQ], f32, tag="mm")
                    toff = 128 * tt - TQ * tq
                    for kc in range(NKC):
                        nc.tensor.matmul(
                            psv[:, 0:256],
                            lhsT=xts[:, kc, toff : toff + 128],
                            rhs=wqkv_sb[:, kc, 512:768],
                            start=(kc == 0),
                            stop=(kc == NKC - 1),
                        )
                    # [128, 256] psum -> [128, 4, 64] slot of v_sb (head-strided)
                    nc.vector.tensor_copy(v_sb[:, tt, :, 0:64], psv[:, 0:256])

            def qkv_units(tq):
                """qkv projection for t-slice tq as filler closures (one psum
                group each) interleaved into the attention instruction stream
                so the PE has independent work while Act runs exp."""
                tqs = slice(TQ * tq, TQ * (tq + 1))
                xts = xts_pool.tile([128, NKC, TQ], bf16, tag="xts")

                def load():
                    nc.sync.dma_start(xts[:], xt_r[:, :, tqs])

                def qk_unit(cp):
                    def emit():
                        ps = ps_mm.tile([128, TQ], f32, tag="mm")
                        for kc in range(NKC):
                            nc.tensor.matmul(
                                ps[:],
                                lhsT=wqkv_sb[:, kc, 128 * cp : 128 * (cp + 1)],
                                rhs=xts[:, kc, :],
                                start=(kc == 0),
                                stop=(kc == NKC - 1),
                            )
                        nc.vector.tensor_scalar_add(
                            qkT_sb[:, cp, tqs], ps[:], bqk_sb[:, cp : cp + 1]
                        )
                    return emit

                def v_unit(tt):
                    def emit():
                        psv = ps_mm.tile([128, TQ], f32, tag="mm")
                        toff = 128 * tt - TQ * tq
                        for kc in range(NKC):
                            nc.tensor.matmul(
                                psv[:, 0:256],
                                lhsT=xts[:, kc, toff : toff + 128],
                                rhs=wqkv_sb[:, kc, 512:768],
                                start=(kc == 0),
                                stop=(kc == NKC - 1),
                            )
                        nc.vector.tensor_copy(v_sb[:, tt, :, 0:64], psv[:, 0:256])
                    return emit

                return load, [qk_unit(cp) for cp in range(4)] + [
                    v_unit(tt) for tt in range(4 * tq, 4 * tq + 4)
                ]

            def proj_units_qb(tq, qb):
                tt = 4 * tq + qb
                units = []
                for nt in range(2):
                    def emit(tt=tt, nt=nt):
                        ts_ = slice(128 * tt, 128 * (tt + 1))
                        ns = slice(512 * nt, 512 * (nt + 1))
                        pso = ps_mm.tile([128, TQ], f32, tag="mm")
                        for hc in range(2):
                            nc.tensor.matmul(
                                pso[:],
                                lhsT=attT_sb[:, hc, ts_],
                                rhs=wproj_sb[:, hc, ns],
                                start=(hc == 0),
                                stop=(hc == 1),
                            )
                        ot = ot_pool.tile([128, TQ], bf16, tag="ot")
                        nc.vector.tensor_copy(ot[:], pso[:])
                        nc.sync.dma_start(out_d[ts_, ns], ot[:])
                    units.append(emit)
                return units

            def proj_units(tq):
                units = []
                for qb in range(4):
                    units.extend(proj_units_qb(tq, qb))
                return units

            def emit_att(tq, fillers, last=False):
                """Attention for tq with PV lagging scores by one step and
                filler matmul units spliced between, so the PE never idles on
                the exp (Act) latency. Each query block's normalize/transpose
                chain is emitted as soon as its diagonal block completes; on
                the last tq the projection units are appended to the filler
                queue the same way, collapsing the pipeline tail."""
                ntk = 4 * tq + 4
                attps = [
                    ps_att.tile([128, HPC, 65], f32, tag="att", name=f"att{tq}_{qb}")
                    for qb in range(4)
                ]
                steps = [(tk, h) for tk in range(ntk) for h in range(HPC)]
                pts = {}

                def emit_sc(i):
                    tk, h = steps[i]
                    d = tk - 4 * tq
                    q0 = 128 * d if d >= 0 else 0
                    w = TQ - q0
                    ks = slice(128 * tk, 128 * (tk + 1))
                    qs = slice(TQ * tq + q0, TQ * (tq + 1))
                    qc, kc_, pr = h // 2, 2 + h // 2, 64 * (h % 2)
                    sc = ps_sc.tile([128, TQ], f32, tag="sc")
                    nc.tensor.matmul(
                        sc[:, 0:w],
                        lhsT=qkT_sb[pr : pr + 64, kc_, ks],
                        rhs=qkT_sb[pr : pr + 64, qc, qs],
                    )
                    pt = pt_pool.tile([128, TQ], bf16, tag="pt")
                    nc.scalar.activation(pt[:, 0:w], sc[:, 0:w], AF.Exp, scale=SCALE)
                    if d >= 0:
                        # zero strictly-above-diagonal in the leading 128x128
                        # block: keep where (j - p) >= 0
                        nc.gpsimd.affine_select(
                            pt[:, 0:128],
                            pt[:, 0:128],
                            pattern=[[1, 128]],
                            compare_op=ALU.is_ge,
                            fill=0.0,
                            base=0,
                            channel_multiplier=-1,
                        )
                    pts[i] = pt

                def emit_pv(i):
                    tk, h = steps[i]
                    d = tk - 4 * tq
                    q0 = 128 * d if d >= 0 else 0
                    pt = pts.pop(i)
                    for qb in range(max(d, 0), 4):
                        qoff = 128 * qb - q0
                        nc.tensor.matmul(
                            attps[qb][:, h, 0:65],
                            lhsT=pt[:, qoff : qoff + 128],
                            rhs=v_sb[:, tk, h, 0:65],
                            start=(tk == 0 and h == 0),
                            stop=(tk == 4 * tq + qb and h == HPC - 1),
                            skip_group_check=True,
                        )

                def emit_norm(qb):
                    rec = rec_pool.tile([128, HPC], f32, tag="rec")
                    nc.vector.reciprocal_approx_fast(
                        out=rec[:], in_=attps[qb][:, :, 64]
                    )
                    att_sb = atts_pool.tile([128, HPC, 64], bf16, tag="atts")
                    for h in range(HPC):
                        nc.vector.tensor_scalar_mul(
                            att_sb[:, h, :], attps[qb][:, h, 0:64], rec[:, h : h + 1]
                        )
                    # transpose on the PE (att_sb [q, hd] -> attT [hd, q]):
                    # ~53ns each vs ~2.5us latency for the DMA XBAR route
                    qslice = slice(TQ * tq + 128 * qb, TQ * tq + 128 * (qb + 1))
                    attTps = ps_att.tile([128, 2, 128], bf16, tag="att")
                    for hc in range(2):
                        nc.tensor.matmul(
                            attTps[:, hc, :],
                            lhsT=att_sb[:, 2 * hc : 2 * hc + 2, :],
                            rhs=ident_sb[:],
                            is_transpose=True,
                            start=(hc == 0),
                            stop=(hc == 1),
                            skip_group_check=True,
                        )
                    nc.vector.tensor_copy(attT_sb[:, :, qslice], attTps[:])

                # PV lags scores by LAG steps so the exp(Act) + mask(Pool)
                # latency is hidden behind later scores/filler matmuls.
                # Dynamically appended fillers (last-tq proj units) are held
                # for DELAY steps: the normalize->transpose->proj readiness
                # chain is ~4us, so scheduling them early just stalls the PE.
                LAG = 4
                DELAY = 10
                nf_est = len(fillers) + (8 if last else 0)
                ns = len(steps)
                fillers = [(0, f) for f in fillers]
                fi = 0
                for i in range(ns + LAG):
                    if i < ns:
                        emit_sc(i)
                    while (fi < len(fillers)
                           and fi < ((i + 1) * nf_est) // ns
                           and fillers[fi][0] <= i):
                        fillers[fi][1]()
                        fi += 1
                    j = i - LAG
                    if j >= 0:
                        emit_pv(j)
                        tk, h = steps[j]
                        if h == HPC - 1 and tk - 4 * tq >= 0:
                            qb = tk - 4 * tq
                            emit_norm(qb)
                            if last:
                                fillers.extend(
                                    (i + DELAY, f) for f in proj_units_qb(tq, qb)
                                )
                while fi < len(fillers):
                    fillers[fi][1]()
                    fi += 1

            # software pipeline: qkv(0) as a prologue; then per tq the
            # attention stream carries qkv(tq+1) + proj(tq-1) matmuls as
            # fillers between its latency-bound sc->exp->PV steps.
            emit_qkv(0)
            for tq in range(NTQ):
                fillers = []
                if tq + 1 < NTQ:
                    load, units = qkv_units(tq + 1)
                    load()
                    fillers += units
                if tq >= 1:
                    fillers += proj_units(tq - 1)
                emit_att(tq, fillers, last=(tq == NTQ - 1))

    nc.compile()
    return nc


def _shard_inputs(x, w_qkv, b_qkv, w_proj, b_proj):
    """Full inputs -> per-core input maps. Core c = (batch b=c//4, group g=c%4)."""
    in_maps = []
    xts = [np.ascontiguousarray(x[b].T).astype(ml_dtypes.bfloat16) for b in range(B)]
    for core in range(NCORES):
        b, g = divmod(core, 4)
        qs = slice(256 * g, 256 * (g + 1))
        ks = slice(C + 256 * g, C + 256 * (g + 1))
        vs = slice(2 * C + 256 * g, 2 * C + 256 * (g + 1))
        wqkv = np.concatenate(
            [w_qkv[:, qs], w_qkv[:, ks], w_qkv[:, vs]], axis=1
        ).astype(ml_dtypes.bfloat16)
        bq, bk = b_qkv[qs], b_qkv[ks]
        bqk = np.ascontiguousarray(
            np.stack([bq[0:128], bq[128:256], bk[0:128], bk[128:256]], axis=1)
        ).astype(np.float32)
        wproj = np.ascontiguousarray(w_proj[256 * g : 256 * (g + 1), :]).astype(
            ml_dtypes.bfloat16
        )
        in_maps.append(
            {"xt": xts[b], "wqkv": np.ascontiguousarray(wqkv), "bqk": bqk,
             "wproj": wproj}
        )
    return in_maps


def kernel(x, w_qkv, b_qkv, w_proj, b_proj):
    x = np.asarray(x, dtype=np.float32)
    w_qkv = np.asarray(w_qkv, dtype=np.float32)
    b_qkv = np.asarray(b_qkv, dtype=np.float32)
    w_proj = np.asarray(w_proj, dtype=np.float32)
    b_proj = np.asarray(b_proj, dtype=np.float32)

    if "nc" not in _CACHE:
        _CACHE["nc"] = build_nc()
    nc = _CACHE["nc"]

    in_maps = _shard_inputs(x, w_qkv, b_qkv, w_proj, b_proj)
    res = run_bass_kernel_spmd(nc, in_maps, list(range(NCORES)))
    # host epilogue: sum head-group partials, add folded bias
    b_eff = (b_qkv[2 * C :].astype(np.float64) @ w_proj.astype(np.float64)
             + b_proj).astype(np.float32)
    out = np.empty((B, T, C), dtype=np.float32)
    for b in range(B):
        acc = res.results[4 * b]["out"].astype(np.float32)
        for g in range(1, 4):
            acc = acc + res.results[4 * b + g]["out"].astype(np.float32)
        out[b] = acc + b_eff
    return out


# revision 4
# speedup vs baseline: 1.1128x; 1.0093x over previous
"""Multi-head causal self-attention (B=2, T=2048, C=1024, H=16, D=64) on 8
Trainium2 NeuronCores.

Sharding: data-parallel over batch (2) x tensor-parallel over heads (4 groups
of 4 heads) = 8 shards, no cross-core communication. Host sums the 4 partial
outputs per batch and adds the bias.

All matmuls in bf16 (1 cycle/row in the PE at any N). Per core:
  qkT = wqk.T @ xT            [4x128, T]   (chunks: q01 q23 k01 k23)
  v   = xT.T @ wv             [T, 4, 64]+ones col (rhs layout for PV)
  per (tk 128-key block, head): scT = k_blk.T @ qT -> exp -> pt [keys, queries]
  PV non-transposed: att[q, h, 0:65] += pt[:, qblk].T @ v[:, tk, h, 0:65]
     (col 64 = ones -> per-query softmax denominator on the PSUM partition)
  normalize per qblock with per-partition reciprocal (DVE), write att bf16
  attT via DMA XBAR transpose (runs on DMA engines, not PE)
  out = attT.T @ wproj  -> bf16 out DMA; host adds b_proj + bv@wproj.

v1 did PV in transposed orientation ([65, w] out, N=w cost) plus PE broadcast
matmuls and activation copies for the denominators; v2's orientation makes PV
cost N=65 per key-block (2x fewer PE cycles) and normalization nearly free.
"""

import numpy as np
import ml_dtypes

import concourse.bass as bass
import concourse.mybir as mybir
import concourse.tile as tile
from concourse import bacc
from concourse.bass_utils import run_bass_kernel_spmd

f32 = mybir.dt.float32
bf16 = mybir.dt.bfloat16
AF = mybir.ActivationFunctionType
ALU = mybir.AluOpType

B, T, C, H, D = 2, 2048, 1024, 16, 64
HPC = 4          # heads per core
NCORES = 8
TQ = 512         # query tile of the attention outer loop
NTQ = T // TQ    # 4
NKC = C // 128   # 8 contraction chunks for the qkv projection
NTT = T // 128   # 16 query 128-blocks
SCALE = 1.0 / 8.0  # 1/sqrt(D)

_CACHE = {}


def build_nc():
    nc = bacc.Bacc("TRN2", target_bir_lowering=False, debug=False)

    xt_d = nc.dram_tensor("xt", [C, T], bf16, kind="ExternalInput")
    wqkv_d = nc.dram_tensor("wqkv", [C, 768], bf16, kind="ExternalInput")
    bqk_d = nc.dram_tensor("bqk", [128, 4], f32, kind="ExternalInput")
    wproj_d = nc.dram_tensor("wproj", [256, C], bf16, kind="ExternalInput")
    out_d = nc.dram_tensor("out", [T, C], bf16, kind="ExternalOutput")

    with tile.TileContext(nc) as tc:
        with (
            tc.tile_pool(name="const", bufs=1) as const,
            tc.tile_pool(name="xts", bufs=3) as xts_pool,
            tc.tile_pool(name="pt", bufs=16) as pt_pool,
            tc.tile_pool(name="atts", bufs=8) as atts_pool,
            tc.tile_pool(name="rec", bufs=8) as rec_pool,
            tc.tile_pool(name="ot", bufs=8) as ot_pool,
            tc.tile_pool(name="ps_mm", bufs=2, space="PSUM") as ps_mm,
            tc.tile_pool(name="ps_sc", bufs=2, space="PSUM") as ps_sc,
            tc.tile_pool(name="ps_att", bufs=4, space="PSUM") as ps_att,
        ):
            # ---- resident tensors; DMAs chunked so compute starts early ----
            wqkv_sb = const.tile([128, NKC, 768], bf16, tag="wqkv")
            wqkv_r = wqkv_d.rearrange("(o p) n -> p o n", p=128)
            nc.scalar.dma_start(wqkv_sb[:, 0, :], wqkv_r[:, 0, :])
            nc.scalar.dma_start(wqkv_sb[:, 1:4, :], wqkv_r[:, 1:4, :])
            nc.scalar.dma_start(wqkv_sb[:, 4:NKC, :], wqkv_r[:, 4:NKC, :])
            bqk_sb = const.tile([128, 4], f32, tag="bqk")
            nc.scalar.dma_start(bqk_sb[:], bqk_d[:, :])
            wproj_sb = const.tile([128, 2, C], bf16, tag="wproj")
            nc.scalar.dma_start(wproj_sb[:], wproj_d.rearrange("(o p) n -> p o n", p=128))

            # qkT chunks: 0 = qT heads01, 1 = qT heads23, 2 = kT h01, 3 = kT h23
            qkT_sb = const.tile([128, 4, T], bf16, tag="qkT")
            # v in PV-rhs layout: [key mod 128, key block, head, 64 vdims + one]
            v_sb = const.tile([128, NTT, HPC, 65], bf16, tag="v")
            nc.vector.memset(v_sb[:, :, :, 64:65], 1.0)
            # attT: chunk hc: partitions = head-dims of heads (2hc, 2hc+1)
            attT_sb = const.tile([128, 2, T], bf16, tag="attT")
            # identity for PE-transpose of the normalized attention
            ident_sb = const.tile([128, 128], bf16, tag="ident")
            nc.vector.memset(ident_sb[:], 1.0)
            nc.gpsimd.affine_select(
                ident_sb[:],
                ident_sb[:],
                pattern=[[1, 128]],
                compare_op=ALU.is_equal,
                fill=0.0,
                base=0,
                channel_multiplier=-1,
            )
            # lower-triangular causal mask (keep j >= p), applied to diagonal
            # blocks with a DVE multiply (lower latency than gpsimd select)
            tri_sb = const.tile([128, 128], bf16, tag="tri")
            nc.vector.memset(tri_sb[:], 1.0)
            nc.gpsimd.affine_select(
                tri_sb[:],
                tri_sb[:],
                pattern=[[1, 128]],
                compare_op=ALU.is_ge,
                fill=0.0,
                base=0,
                channel_multiplier=-1,
            )

            xt_r = xt_d.rearrange("(o p) t -> p o t", p=128)

            def emit_qkv(tq):
                tqs = slice(TQ * tq, TQ * (tq + 1))
                xts = xts_pool.tile([128, NKC, TQ], bf16, tag="xts")
                nc.sync.dma_start(xts[:, 0, :], xt_r[:, 0, tqs])
                nc.sync.dma_start(xts[:, 1:4, :], xt_r[:, 1:4, tqs])
                nc.sync.dma_start(xts[:, 4:NKC, :], xt_r[:, 4:NKC, tqs])
                # q,k transposed: psum = wqkv_chunk.T @ xT
                if tq == 0:
                    # kc-major over pairs of open psum groups to hide DMA ramp
                    for cpp in range(2):
                        ps_pair = [
                            ps_mm.tile([128, TQ], f32, tag="mm", name=f"qk{cpp}{i}")
                            for i in range(2)
                        ]
                        for kc in range(NKC):
                            for i in range(2):
                                cp = 2 * cpp + i
                                nc.tensor.matmul(
                                    ps_pair[i][:],
                                    lhsT=wqkv_sb[:, kc, 128 * cp : 128 * (cp + 1)],
                                    rhs=xts[:, kc, :],
                                    start=(kc == 0),
                                    stop=(kc == NKC - 1),
                                    skip_group_check=True,
                                )
                        for i in range(2):
                            cp = 2 * cpp + i
                            nc.vector.tensor_scalar_add(
                                qkT_sb[:, cp, tqs], ps_pair[i][:], bqk_sb[:, cp : cp + 1]
                            )
                else:
                    for cp in range(4):
                        ps = ps_mm.tile([128, TQ], f32, tag="mm")
                        for kc in range(NKC):
                            nc.tensor.matmul(
                                ps[:],
                                lhsT=wqkv_sb[:, kc, 128 * cp : 128 * (cp + 1)],
                                rhs=xts[:, kc, :],
                                start=(kc == 0),
                                stop=(kc == NKC - 1),
                            )
                        nc.vector.tensor_scalar_add(
                            qkT_sb[:, cp, tqs], ps[:], bqk_sb[:, cp : cp + 1]
                        )
                # v: psum = xT_chunk.T @ wv  (no bias: folded into host output)
                for tt in range(4 * tq, 4 * tq + 4):
                    psv = ps_mm.tile([128, TQ], f32, tag="mm")
                    toff = 128 * tt - TQ * tq
                    for kc in range(NKC):
                        nc.tensor.matmul(
                            psv[:, 0:256],
                            lhsT=xts[:, kc, toff : toff + 128],
                            rhs=wqkv_sb[:, kc, 512:768],
                            start=(kc == 0),
                            stop=(kc == NKC - 1),
                        )
                    # [128, 256] psum -> [128, 4, 64] slot of v_sb (head-strided)
                    nc.vector.tensor_copy(v_sb[:, tt, :, 0:64], psv[:, 0:256])

            def qkv_units(tq):
                """qkv projection for t-slice tq as filler closures (one psum
                group each) interleaved into the attention instruction stream
                so the PE has independent work while Act runs exp."""
                tqs = slice(TQ * tq, TQ * (tq + 1))
                xts = xts_pool.tile([128, NKC, TQ], bf16, tag="xts")

                def load():
                    nc.sync.dma_start(xts[:, 0:4, :], xt_r[:, 0:4, tqs])
                    nc.sync.dma_start(xts[:, 4:NKC, :], xt_r[:, 4:NKC, tqs])

                def qk_unit(cp):
                    def emit():
                        ps = ps_mm.tile([128, TQ], f32, tag="mm")
                        for kc in range(NKC):
                            nc.tensor.matmul(
                                ps[:],
                                lhsT=wqkv_sb[:, kc, 128 * cp : 128 * (cp + 1)],
                                rhs=xts[:, kc, :],
                                start=(kc == 0),
                                stop=(kc == NKC - 1),
                            )
                        nc.vector.tensor_scalar_add(
                            qkT_sb[:, cp, tqs], ps[:], bqk_sb[:, cp : cp + 1]
                        )
                    return emit

                def v_unit(tt):
                    def emit():
                        psv = ps_mm.tile([128, TQ], f32, tag="mm")
                        toff = 128 * tt - TQ * tq
                        for kc in range(NKC):
                            nc.tensor.matmul(
                                psv[:, 0:256],
                                lhsT=xts[:, kc, toff : toff + 128],
                                rhs=wqkv_sb[:, kc, 512:768],
                                start=(kc == 0),
                                stop=(kc == NKC - 1),
                            )
                        nc.vector.tensor_copy(v_sb[:, tt, :, 0:64], psv[:, 0:256])
                    return emit

                return load, [qk_unit(cp) for cp in range(4)] + [
                    v_unit(tt) for tt in range(4 * tq, 4 * tq + 4)
                ]

            def proj_units_qb(tq, qb):
                tt = 4 * tq + qb
                ot = [None]

                def emit(nt):
                    ts_ = slice(128 * tt, 128 * (tt + 1))
                    ns = slice(512 * nt, 512 * (nt + 1))
                    pso = ps_mm.tile([128, TQ], f32, tag="mm")
                    for hc in range(2):
                        nc.tensor.matmul(
                            pso[:],
                            lhsT=attT_sb[:, hc, ts_],
                            rhs=wproj_sb[:, hc, ns],
                            start=(hc == 0),
                            stop=(hc == 1),
                        )
                    if nt == 0:
                        ot[0] = ot_pool.tile(
                            [128, 2, TQ], bf16, tag="ot", name=f"ot{tt}"
                        )
                    nc.vector.tensor_copy(ot[0][:, nt, :], pso[:])
                    if nt == 1:
                        # one merged DMA per 128-row block (fewer DMAs =
                        # less serialization on the single-slot HWDGE)
                        nc.sync.dma_start(out_d[ts_, :], ot[0][:])

                return [lambda: emit(0), lambda: emit(1)]

            def proj_units(tq):
                units = []
                for qb in range(4):
                    units.extend(proj_units_qb(tq, qb))
                return units

            def emit_att(tq, fillers, last=False):
                """Attention for tq with PV lagging scores by one step and
                filler matmul units spliced between, so the PE never idles on
                the exp (Act) latency. Each query block's normalize/transpose
                chain is emitted as soon as its diagonal block completes; on
                the last tq the projection units are appended to the filler
                queue the same way, collapsing the pipeline tail."""
                ntk = 4 * tq + 4
                attps = [
                    ps_att.tile([128, HPC, 65], f32, tag="att", name=f"att{tq}_{qb}")
                    for qb in range(4)
                ]
                steps = [(tk, h) for tk in range(ntk) for h in range(HPC)]
                pts = {}

                def emit_sc(i):
                    tk, h = steps[i]
                    d = tk - 4 * tq
                    q0 = 128 * d if d >= 0 else 0
                    w = TQ - q0
                    ks = slice(128 * tk, 128 * (tk + 1))
                    qs = slice(TQ * tq + q0, TQ * (tq + 1))
                    qc, kc_, pr = h // 2, 2 + h // 2, 64 * (h % 2)
                    sc = ps_sc.tile([128, TQ], f32, tag="sc")
                    nc.tensor.matmul(
                        sc[:, 0:w],
                        lhsT=qkT_sb[pr : pr + 64, kc_, ks],
                        rhs=qkT_sb[pr : pr + 64, qc, qs],
                    )
                    pt = pt_pool.tile([128, TQ], bf16, tag="pt")
                    nc.scalar.activation(pt[:, 0:w], sc[:, 0:w], AF.Exp, scale=SCALE)
                    if d >= 0:
                        nc.vector.tensor_mul(pt[:, 0:128], pt[:, 0:128], tri_sb[:])
                    pts[i] = pt

                def emit_pv(i):
                    tk, h = steps[i]
                    d = tk - 4 * tq
                    q0 = 128 * d if d >= 0 else 0
                    pt = pts.pop(i)
                    for qb in range(max(d, 0), 4):
                        qoff = 128 * qb - q0
                        nc.tensor.matmul(
                            attps[qb][:, h, 0:65],
                            lhsT=pt[:, qoff : qoff + 128],
                            rhs=v_sb[:, tk, h, 0:65],
                            start=(tk == 0 and h == 0),
                            stop=(tk == 4 * tq + qb and h == HPC - 1),
                            skip_group_check=True,
                        )

                def emit_norm(qb):
                    rec = rec_pool.tile([128, HPC], f32, tag="rec")
                    nc.vector.reciprocal_approx_fast(
                        out=rec[:], in_=attps[qb][:, :, 64]
                    )
                    att_sb = atts_pool.tile([128, HPC, 64], bf16, tag="atts")
                    for h in range(HPC):
                        nc.vector.tensor_scalar_mul(
                            att_sb[:, h, :], attps[qb][:, h, 0:64], rec[:, h : h + 1]
                        )
                    # transpose on the PE (att_sb [q, hd] -> attT [hd, q]):
                    # ~53ns each vs ~2.5us latency for the DMA XBAR route
                    qslice = slice(TQ * tq + 128 * qb, TQ * tq + 128 * (qb + 1))
                    attTps = ps_att.tile([128, 2, 128], bf16, tag="att")
                    for hc in range(2):
                        nc.tensor.matmul(
                            attTps[:, hc, :],
                            lhsT=att_sb[:, 2 * hc : 2 * hc + 2, :],
                            rhs=ident_sb[:],
                            is_transpose=True,
                            start=(hc == 0),
                            stop=(hc == 1),
                            skip_group_check=True,
                        )
                    nc.vector.tensor_copy(attT_sb[:, :, qslice], attTps[:])

                # PV lags scores by LAG steps so the exp(Act) + mask(DVE)
                # latency is hidden behind later scores/filler matmuls.
                # Dynamically appended fillers (last-tq proj units) are held
                # for DELAY steps: the normalize->transpose->proj readiness
                # chain is long, so scheduling them early just stalls the PE.
                LAG = 6
                DELAY = 10
                nf_est = len(fillers) + (8 if last else 0)
                ns = len(steps)
                fillers = [(0, f) for f in fillers]
                fi = 0
                for i in range(ns + LAG):
                    if i < ns:
                        emit_sc(i)
                    while (fi < len(fillers)
                           and fi < ((i + 1) * nf_est) // ns
                           and fillers[fi][0] <= i):
                        fillers[fi][1]()
                        fi += 1
                    j = i - LAG
                    if j >= 0:
                        emit_pv(j)
                        tk, h = steps[j]
                        if h == HPC - 1 and tk - 4 * tq >= 0:
                            qb = tk - 4 * tq
                            emit_norm(qb)
                            if last:
                                fillers.extend(
                                    (i + DELAY, f) for f in proj_units_qb(tq, qb)
                                )
                while fi < len(fillers):
                    fillers[fi][1]()
                    fi += 1

            # software pipeline: qkv(0) as a prologue; the per-tq attention
            # streams carry the remaining qkv/proj matmuls as fillers,
            # distributed by each attention block's Act-vs-PE deficit (the
            # later blocks are increasingly exp-bound, so all proj work is
            # pushed toward them; qkv(t) must complete before att(t) starts).
            emit_qkv(0)
            loads = {}
            plan = {0: [], 1: [], 2: [], 3: []}
            for t in (1, 2, 3):
                load, units = qkv_units(t)
                loads[t - 1] = load
                plan[t - 1] += units
            plan[2] += proj_units(0)
            plan[3] += proj_units(1) + proj_units(2)
            for tq in range(NTQ):
                if tq in loads:
                    loads[tq]()
                emit_att(tq, plan[tq], last=(tq == NTQ - 1))

    nc.compile()
    return nc


def _shard_inputs(x, w_qkv, b_qkv, w_proj, b_proj):
    """Full inputs -> per-core input maps. Core c = (batch b=c//4, group g=c%4)."""
    in_maps = []
    xts = [np.ascontiguousarray(x[b].T).astype(ml_dtypes.bfloat16) for b in range(B)]
    for core in range(NCORES):
        b, g = divmod(core, 4)
        qs = slice(256 * g, 256 * (g + 1))
        ks = slice(C + 256 * g, C + 256 * (g + 1))
        vs = slice(2 * C + 256 * g, 2 * C + 256 * (g + 1))
        wqkv = np.concatenate(
            [w_qkv[:, qs], w_qkv[:, ks], w_qkv[:, vs]], axis=1
        ).astype(ml_dtypes.bfloat16)
        bq, bk = b_qkv[qs], b_qkv[ks]
        bqk = np.ascontiguousarray(
            np.stack([bq[0:128], bq[128:256], bk[0:128], bk[128:256]], axis=1)
        ).astype(np.float32)
        wproj = np.ascontiguousarray(w_proj[256 * g : 256 * (g + 1), :]).astype(
            ml_dtypes.bfloat16
        )
        in_maps.append(
            {"xt": xts[b], "wqkv": np.ascontiguousarray(wqkv), "bqk": bqk,
             "wproj": wproj}
        )
    return in_maps


def kernel(x, w_qkv, b_qkv, w_proj, b_proj):
    x = np.asarray(x, dtype=np.float32)
    w_qkv = np.asarray(w_qkv, dtype=np.float32)
    b_qkv = np.asarray(b_qkv, dtype=np.float32)
    w_proj = np.asarray(w_proj, dtype=np.float32)
    b_proj = np.asarray(b_proj, dtype=np.float32)

    if "nc" not in _CACHE:
        _CACHE["nc"] = build_nc()
    nc = _CACHE["nc"]

    in_maps = _shard_inputs(x, w_qkv, b_qkv, w_proj, b_proj)
    res = run_bass_kernel_spmd(nc, in_maps, list(range(NCORES)))
    # host epilogue: sum head-group partials, add folded bias
    b_eff = (b_qkv[2 * C :].astype(np.float64) @ w_proj.astype(np.float64)
             + b_proj).astype(np.float32)
    out = np.empty((B, T, C), dtype=np.float32)
    for b in range(B):
        acc = res.results[4 * b]["out"].astype(np.float32)
        for g in range(1, 4):
            acc = acc + res.results[4 * b + g]["out"].astype(np.float32)
        out[b] = acc + b_eff
    return out


# revision 5
# speedup vs baseline: 1.1231x; 1.0092x over previous
"""Multi-head causal self-attention (B=2, T=2048, C=1024, H=16, D=64) on 8
Trainium2 NeuronCores.

Sharding: data-parallel over batch (2) x tensor-parallel over heads (4 groups
of 4 heads) = 8 shards, no cross-core communication. Host sums the 4 partial
outputs per batch and adds the bias.

All matmuls in bf16 (1 cycle/row in the PE at any N). Per core:
  qkT = wqk.T @ xT            [4x128, T]   (chunks: q01 q23 k01 k23)
  v   = xT.T @ wv             [T, 4, 64]+ones col (rhs layout for PV)
  per (tk 128-key block, head): scT = k_blk.T @ qT -> exp -> pt [keys, queries]
  PV non-transposed: att[q, h, 0:65] += pt[:, qblk].T @ v[:, tk, h, 0:65]
     (col 64 = ones -> per-query softmax denominator on the PSUM partition)
  normalize per qblock with per-partition reciprocal (DVE), write att bf16
  attT via DMA XBAR transpose (runs on DMA engines, not PE)
  out = attT.T @ wproj  -> bf16 out DMA; host adds b_proj + bv@wproj.

v1 did PV in transposed orientation ([65, w] out, N=w cost) plus PE broadcast
matmuls and activation copies for the denominators; v2's orientation makes PV
cost N=65 per key-block (2x fewer PE cycles) and normalization nearly free.
"""

import numpy as np
import ml_dtypes

import concourse.bass as bass
import concourse.mybir as mybir
import concourse.tile as tile
from concourse import bacc
from concourse.bass_utils import run_bass_kernel_spmd

f32 = mybir.dt.float32
bf16 = mybir.dt.bfloat16
AF = mybir.ActivationFunctionType
ALU = mybir.AluOpType

B, T, C, H, D = 2, 2048, 1024, 16, 64
HPC = 4          # heads per core
NCORES = 8
TQ = 512         # query tile of the attention outer loop
NTQ = T // TQ    # 4
NKC = C // 128   # 8 contraction chunks for the qkv projection
NTT = T // 128   # 16 query 128-blocks
SCALE = 1.0 / 8.0  # 1/sqrt(D)

_CACHE = {}


def build_nc():
    nc = bacc.Bacc("TRN2", target_bir_lowering=False, debug=False)

    xt_d = nc.dram_tensor("xt", [C, T], bf16, kind="ExternalInput")
    wqkv_d = nc.dram_tensor("wqkv", [C, 768], bf16, kind="ExternalInput")
    bqk_d = nc.dram_tensor("bqk", [128, 4], f32, kind="ExternalInput")
    wproj_d = nc.dram_tensor("wproj", [256, C], bf16, kind="ExternalInput")
    out_d = nc.dram_tensor("out", [T, C], bf16, kind="ExternalOutput")

    with tile.TileContext(nc) as tc:
        with (
            tc.tile_pool(name="const", bufs=1) as const,
            tc.tile_pool(name="xts", bufs=3) as xts_pool,
            tc.tile_pool(name="pt", bufs=16) as pt_pool,
            tc.tile_pool(name="atts", bufs=8) as atts_pool,
            tc.tile_pool(name="rec", bufs=8) as rec_pool,
            tc.tile_pool(name="ot", bufs=8) as ot_pool,
            tc.tile_pool(name="ps_mm", bufs=2, space="PSUM") as ps_mm,
            tc.tile_pool(name="ps_sc", bufs=2, space="PSUM") as ps_sc,
            tc.tile_pool(name="ps_att", bufs=4, space="PSUM") as ps_att,
        ):
            # ---- resident tensors; DMAs chunked so compute starts early ----
            wqkv_sb = const.tile([128, NKC, 768], bf16, tag="wqkv")
            wqkv_r = wqkv_d.rearrange("(o p) n -> p o n", p=128)
            nc.scalar.dma_start(wqkv_sb[:, 0, :], wqkv_r[:, 0, :])
            nc.scalar.dma_start(wqkv_sb[:, 1:4, :], wqkv_r[:, 1:4, :])
            nc.scalar.dma_start(wqkv_sb[:, 4:NKC, :], wqkv_r[:, 4:NKC, :])
            bqk_sb = const.tile([128, 4], f32, tag="bqk")
            nc.scalar.dma_start(bqk_sb[:], bqk_d[:, :])
            wproj_sb = const.tile([128, 2, C], bf16, tag="wproj")
            nc.scalar.dma_start(wproj_sb[:], wproj_d.rearrange("(o p) n -> p o n", p=128))

            # qkT chunks: 0 = qT heads01, 1 = qT heads23, 2 = kT h01, 3 = kT h23
            qkT_sb = const.tile([128, 4, T], bf16, tag="qkT")
            # v in PV-rhs layout: [key mod 128, key block, head, 64 vdims + one]
            v_sb = const.tile([128, NTT, HPC, 65], bf16, tag="v")
            nc.vector.memset(v_sb[:, :, :, 64:65], 1.0)
            # attT: chunk hc: partitions = head-dims of heads (2hc, 2hc+1)
            attT_sb = const.tile([128, 2, T], bf16, tag="attT")
            # identity for PE-transpose of the normalized attention
            ident_sb = const.tile([128, 128], bf16, tag="ident")
            nc.vector.memset(ident_sb[:], 1.0)
            nc.gpsimd.affine_select(
                ident_sb[:],
                ident_sb[:],
                pattern=[[1, 128]],
                compare_op=ALU.is_equal,
                fill=0.0,
                base=0,
                channel_multiplier=-1,
            )
            # lower-triangular causal mask (keep j >= p), applied to diagonal
            # blocks with a DVE multiply (lower latency than gpsimd select)
            tri_sb = const.tile([128, 128], bf16, tag="tri")
            nc.vector.memset(tri_sb[:], 1.0)
            nc.gpsimd.affine_select(
                tri_sb[:],
                tri_sb[:],
                pattern=[[1, 128]],
                compare_op=ALU.is_ge,
                fill=0.0,
                base=0,
                channel_multiplier=-1,
            )

            xt_r = xt_d.rearrange("(o p) t -> p o t", p=128)

            def emit_qkv(tq):
                tqs = slice(TQ * tq, TQ * (tq + 1))
                xts = xts_pool.tile([128, NKC, TQ], bf16, tag="xts")
                nc.sync.dma_start(xts[:, 0, :], xt_r[:, 0, tqs])
                nc.sync.dma_start(xts[:, 1:4, :], xt_r[:, 1:4, tqs])
                nc.sync.dma_start(xts[:, 4:NKC, :], xt_r[:, 4:NKC, tqs])
                # q,k transposed: psum = wqkv_chunk.T @ xT
                if tq == 0:
                    # kc-major over pairs of open psum groups to hide DMA ramp
                    for cpp in range(2):
                        ps_pair = [
                            ps_mm.tile([128, TQ], f32, tag="mm", name=f"qk{cpp}{i}")
                            for i in range(2)
                        ]
                        for kc in range(NKC):
                            for i in range(2):
                                cp = 2 * cpp + i
                                nc.tensor.matmul(
                                    ps_pair[i][:],
                                    lhsT=wqkv_sb[:, kc, 128 * cp : 128 * (cp + 1)],
                                    rhs=xts[:, kc, :],
                                    start=(kc == 0),
                                    stop=(kc == NKC - 1),
                                    skip_group_check=True,
                                )
                        for i in range(2):
                            cp = 2 * cpp + i
                            nc.vector.tensor_scalar_add(
                                qkT_sb[:, cp, tqs], ps_pair[i][:], bqk_sb[:, cp : cp + 1]
                            )
                else:
                    for cp in range(4):
                        ps = ps_mm.tile([128, TQ], f32, tag="mm")
                        for kc in range(NKC):
                            nc.tensor.matmul(
                                ps[:],
                                lhsT=wqkv_sb[:, kc, 128 * cp : 128 * (cp + 1)],
                                rhs=xts[:, kc, :],
                                start=(kc == 0),
                                stop=(kc == NKC - 1),
                            )
                        nc.vector.tensor_scalar_add(
                            qkT_sb[:, cp, tqs], ps[:], bqk_sb[:, cp : cp + 1]
                        )
                # v: psum = xT_chunk.T @ wv  (no bias: folded into host output)
                for tt in range(4 * tq, 4 * tq + 4):
                    psv = ps_mm.tile([128, TQ], f32, tag="mm")
                    toff = 128 * tt - TQ * tq
                    for kc in range(NKC):
                        nc.tensor.matmul(
                            psv[:, 0:256],
                            lhsT=xts[:, kc, toff : toff + 128],
                            rhs=wqkv_sb[:, kc, 512:768],
                            start=(kc == 0),
                            stop=(kc == NKC - 1),
                        )
                    # [128, 256] psum -> [128, 4, 64] slot of v_sb (head-strided)
                    nc.vector.tensor_copy(v_sb[:, tt, :, 0:64], psv[:, 0:256])

            def qkv_units(tq):
                """qkv projection for t-slice tq as filler closures (one psum
                group each) interleaved into the attention instruction stream
                so the PE has independent work while Act runs exp."""
                tqs = slice(TQ * tq, TQ * (tq + 1))
                xts = xts_pool.tile([128, NKC, TQ], bf16, tag="xts")

                def load():
                    nc.sync.dma_start(xts[:, 0:4, :], xt_r[:, 0:4, tqs])
                    nc.sync.dma_start(xts[:, 4:NKC, :], xt_r[:, 4:NKC, tqs])

                def qk_unit(cp):
                    def emit():
                        ps = ps_mm.tile([128, TQ], f32, tag="mm")
                        for kc in range(NKC):
                            nc.tensor.matmul(
                                ps[:],
                                lhsT=wqkv_sb[:, kc, 128 * cp : 128 * (cp + 1)],
                                rhs=xts[:, kc, :],
                                start=(kc == 0),
                                stop=(kc == NKC - 1),
                            )
                        nc.vector.tensor_scalar_add(
                            qkT_sb[:, cp, tqs], ps[:], bqk_sb[:, cp : cp + 1]
                        )
                    return emit

                def v_unit(tt):
                    def emit():
                        psv = ps_mm.tile([128, TQ], f32, tag="mm")
                        toff = 128 * tt - TQ * tq
                        for kc in range(NKC):
                            nc.tensor.matmul(
                                psv[:, 0:256],
                                lhsT=xts[:, kc, toff : toff + 128],
                                rhs=wqkv_sb[:, kc, 512:768],
                                start=(kc == 0),
                                stop=(kc == NKC - 1),
                            )
                        nc.vector.tensor_copy(v_sb[:, tt, :, 0:64], psv[:, 0:256])
                    return emit

                return load, [qk_unit(cp) for cp in range(4)] + [
                    v_unit(tt) for tt in range(4 * tq, 4 * tq + 4)
                ]

            def proj_units_qb(tq, qb):
                tt = 4 * tq + qb
                ot = [None]

                def emit(nt):
                    ts_ = slice(128 * tt, 128 * (tt + 1))
                    ns = slice(512 * nt, 512 * (nt + 1))
                    pso = ps_mm.tile([128, TQ], f32, tag="mm")
                    for hc in range(2):
                        nc.tensor.matmul(
                            pso[:],
                            lhsT=attT_sb[:, hc, ts_],
                            rhs=wproj_sb[:, hc, ns],
                            start=(hc == 0),
                            stop=(hc == 1),
                        )
                    if nt == 0:
                        ot[0] = ot_pool.tile(
                            [128, 2, TQ], bf16, tag="ot", name=f"ot{tt}"
                        )
                    nc.vector.tensor_copy(ot[0][:, nt, :], pso[:])
                    if nt == 1:
                        # one merged DMA per 128-row block (fewer DMAs =
                        # less serialization on the single-slot HWDGE)
                        nc.sync.dma_start(out_d[ts_, :], ot[0][:])

                return [lambda: emit(0), lambda: emit(1)]

            def proj_units(tq):
                units = []
                for qb in range(4):
                    units.extend(proj_units_qb(tq, qb))
                return units

            def emit_att(tq, fillers, last=False):
                """Attention for tq with PV lagging scores by one step and
                filler matmul units spliced between, so the PE never idles on
                the exp (Act) latency. Each query block's normalize/transpose
                chain is emitted as soon as its diagonal block completes; on
                the last tq the projection units are appended to the filler
                queue the same way, collapsing the pipeline tail."""
                ntk = 4 * tq + 4
                attps = [
                    ps_att.tile([128, HPC, 65], f32, tag="att", name=f"att{tq}_{qb}")
                    for qb in range(4)
                ]
                steps = [(tk, h) for tk in range(ntk) for h in range(HPC)]
                pts = {}

                def emit_sc(i):
                    tk, h = steps[i]
                    d = tk - 4 * tq
                    q0 = 128 * d if d >= 0 else 0
                    w = TQ - q0
                    ks = slice(128 * tk, 128 * (tk + 1))
                    qs = slice(TQ * tq + q0, TQ * (tq + 1))
                    qc, kc_, pr = h // 2, 2 + h // 2, 64 * (h % 2)
                    sc = ps_sc.tile([128, TQ], f32, tag="sc")
                    nc.tensor.matmul(
                        sc[:, 0:w],
                        lhsT=qkT_sb[pr : pr + 64, kc_, ks],
                        rhs=qkT_sb[pr : pr + 64, qc, qs],
                    )
                    pt = pt_pool.tile([128, TQ], bf16, tag="pt")
                    nc.scalar.activation(pt[:, 0:w], sc[:, 0:w], AF.Exp, scale=SCALE)
                    if d >= 0:
                        nc.vector.tensor_mul(pt[:, 0:128], pt[:, 0:128], tri_sb[:])
                    pts[i] = pt

                def emit_pv(i):
                    tk, h = steps[i]
                    d = tk - 4 * tq
                    q0 = 128 * d if d >= 0 else 0
                    pt = pts.pop(i)
                    for qb in range(max(d, 0), 4):
                        qoff = 128 * qb - q0
                        nc.tensor.matmul(
                            attps[qb][:, h, 0:65],
                            lhsT=pt[:, qoff : qoff + 128],
                            rhs=v_sb[:, tk, h, 0:65],
                            start=(tk == 0 and h == 0),
                            stop=(tk == 4 * tq + qb and h == HPC - 1),
                            skip_group_check=True,
                        )

                def emit_norm(qb):
                    rec = rec_pool.tile([128, HPC], f32, tag="rec")
                    nc.vector.reciprocal_approx_fast(
                        out=rec[:], in_=attps[qb][:, :, 64]
                    )
                    att_sb = atts_pool.tile([128, HPC, 64], bf16, tag="atts")
                    nc.vector.tensor_mul(
                        att_sb[:],
                        attps[qb][:, :, 0:64],
                        rec[:, :, None].broadcast_to([128, HPC, 64]),
                    )
                    # transpose on the PE (att_sb [q, hd] -> attT [hd, q]):
                    # ~53ns each vs ~2.5us latency for the DMA XBAR route
                    qslice = slice(TQ * tq + 128 * qb, TQ * tq + 128 * (qb + 1))
                    attTps = ps_att.tile([128, 2, 128], bf16, tag="att")
                    for hc in range(2):
                        nc.tensor.matmul(
                            attTps[:, hc, :],
                            lhsT=att_sb[:, 2 * hc : 2 * hc + 2, :],
                            rhs=ident_sb[:],
                            is_transpose=True,
                            start=(hc == 0),
                            stop=(hc == 1),
                            skip_group_check=True,
                        )
                    nc.vector.tensor_copy(attT_sb[:, :, qslice], attTps[:])

                # PV lags scores by LAG steps so the exp(Act) + mask(DVE)
                # latency is hidden behind later scores/filler matmuls.
                # Dynamically appended fillers (last-tq proj units) are held
                # for DELAY steps: the normalize->transpose->proj readiness
                # chain is long, so scheduling them early just stalls the PE.
                LAG = 6
                DELAY = 10
                nf_est = len(fillers) + (8 if last else 0)
                ns = len(steps)
                fillers = [(0, f) for f in fillers]
                fi = 0
                for i in range(ns + LAG):
                    if i < ns:
                        emit_sc(i)
                    while (fi < len(fillers)
                           and fi < ((i + 1) * nf_est) // ns
                           and fillers[fi][0] <= i):
                        fillers[fi][1]()
                        fi += 1
                    j = i - LAG
                    if j >= 0:
                        emit_pv(j)
                        tk, h = steps[j]
                        if h == HPC - 1 and tk - 4 * tq >= 0:
                            qb = tk - 4 * tq
                            emit_norm(qb)
                            if last:
                                fillers.extend(
                                    (i + DELAY, f) for f in proj_units_qb(tq, qb)
                                )
                while fi < len(fillers):
                    fillers[fi][1]()
                    fi += 1

            # software pipeline: qkv(0) as a prologue; the per-tq attention
            # streams carry the remaining qkv/proj matmuls as fillers,
            # distributed by each attention block's Act-vs-PE deficit (the
            # later blocks are increasingly exp-bound, so all proj work is
            # pushed toward them; qkv(t) must complete before att(t) starts).
            emit_qkv(0)
            loads = {}
            plan = {0: [], 1: [], 2: [], 3: []}
            for t in (1, 2, 3):
                load, units = qkv_units(t)
                loads[t - 1] = load
                plan[t - 1] += units
            plan[2] += proj_units(0)
            plan[3] += proj_units(1) + proj_units(2)
            for tq in range(NTQ):
                if tq in loads:
                    loads[tq]()
                emit_att(tq, plan[tq], last=(tq == NTQ - 1))

    nc.compile()
    return nc


def _shard_inputs(x, w_qkv, b_qkv, w_proj, b_proj):
    """Full inputs -> per-core input maps. Core c = (batch b=c//4, group g=c%4)."""
    in_maps = []
    xts = [np.ascontiguousarray(x[b].T).astype(ml_dtypes.bfloat16) for b in range(B)]
    for core in range(NCORES):
        b, g = divmod(core, 4)
        qs = slice(256 * g, 256 * (g + 1))
        ks = slice(C + 256 * g, C + 256 * (g + 1))
        vs = slice(2 * C + 256 * g, 2 * C + 256 * (g + 1))
        wqkv = np.concatenate(
            [w_qkv[:, qs], w_qkv[:, ks], w_qkv[:, vs]], axis=1
        ).astype(ml_dtypes.bfloat16)
        bq, bk = b_qkv[qs], b_qkv[ks]
        bqk = np.ascontiguousarray(
            np.stack([bq[0:128], bq[128:256], bk[0:128], bk[128:256]], axis=1)
        ).astype(np.float32)
        wproj = np.ascontiguousarray(w_proj[256 * g : 256 * (g + 1), :]).astype(
            ml_dtypes.bfloat16
        )
        in_maps.append(
            {"xt": xts[b], "wqkv": np.ascontiguousarray(wqkv), "bqk": bqk,
             "wproj": wproj}
        )
    return in_maps


def kernel(x, w_qkv, b_qkv, w_proj, b_proj):
    x = np.asarray(x, dtype=np.float32)
    w_qkv = np.asarray(w_qkv, dtype=np.float32)
    b_qkv = np.asarray(b_qkv, dtype=np.float32)
    w_proj = np.asarray(w_proj, dtype=np.float32)
    b_proj = np.asarray(b_proj, dtype=np.float32)

    if "nc" not in _CACHE:
        _CACHE["nc"] = build_nc()
    nc = _CACHE["nc"]

    in_maps = _shard_inputs(x, w_qkv, b_qkv, w_proj, b_proj)
    res = run_bass_kernel_spmd(nc, in_maps, list(range(NCORES)))
    # host epilogue: sum head-group partials, add folded bias
    b_eff = (b_qkv[2 * C :].astype(np.float64) @ w_proj.astype(np.float64)
             + b_proj).astype(np.float32)
    out = np.empty((B, T, C), dtype=np.float32)
    for b in range(B):
        acc = res.results[4 * b]["out"].astype(np.float32)
        for g in range(1, 4):
            acc = acc + res.results[4 * b + g]["out"].astype(np.float32)
        out[b] = acc + b_eff
    return out


# revision 8
# speedup vs baseline: 1.1511x; 1.0250x over previous
"""Multi-head causal self-attention (B=2, T=2048, C=1024, H=16, D=64) on 8
Trainium2 NeuronCores.

Sharding: data-parallel over batch (2) x tensor-parallel over heads (4 groups
of 4 heads) = 8 shards, no cross-core communication. Host sums the 4 partial
outputs per batch and adds the (folded) bias.

All matmuls in bf16 (1 PE cycle/row at any moving size; fp8/DoubleRow was
evaluated but every fp8 station exceeds the 2e-2 accuracy gate). Per core:
  qkT = wqk.T @ xT            [4x128, T]   (chunks: q01 q23 k01 k23)
  v   = xT.T @ wv             [T, 4, 64]+ones col (rhs layout for PV)
  per (tk 128-key block, head): scT = k_blk.T @ qT -> exp -> pt [keys, queries]
  PV non-transposed: att[q, h, 0:65] += pt[:, qblk].T @ v[:, tk, h, 0:65]
     (col 64 = ones -> per-query softmax denominator lands per PSUM partition,
      so normalization is one reciprocal + one broadcast multiply on the DVE)
  attT via PE transpose (identity matmul; the DMA XBAR route has ~2.5us
     chain latency that stalled the projection)
  out = attT.T @ wproj  -> bf16 out DMA; host adds b_proj + bv@wproj.

The attention inner loop is latency-bound (exp on the scalar engine), so the
qkv projection and output projection are split into filler units appended
AFTER each attention block's steps: the tile scheduler pops ready work by
emission-index priority, so attention (which feeds the saturated Act engine)
always wins ties while fillers absorb every PE stall. qkv(t) fills block
t-1; all projection work fills the final (most exp-bound) block. The causal
mask is a DVE multiply with a precomputed triangle; all bias matmuls are
folded into the host epilogue (softmax rows sum to 1, so the V bias
contributes bv @ w_proj to every output row).
"""

import numpy as np
import ml_dtypes

import concourse.bass as bass
import concourse.mybir as mybir
import concourse.tile as tile
from concourse import bacc
from concourse.bass_utils import run_bass_kernel_spmd

f32 = mybir.dt.float32
bf16 = mybir.dt.bfloat16
AF = mybir.ActivationFunctionType
ALU = mybir.AluOpType

B, T, C, H, D = 2, 2048, 1024, 16, 64
HPC = 4          # heads per core
NCORES = 8
TQ = 512         # query tile of the attention outer loop
NTQ = T // TQ    # 4
NKC = C // 128   # 8 contraction chunks for the qkv projection
NTT = T // 128   # 16 query 128-blocks
SCALE = 1.0 / 8.0  # 1/sqrt(D)

_CACHE = {}


def build_nc():
    nc = bacc.Bacc("TRN2", target_bir_lowering=False, debug=False)

    xt_d = nc.dram_tensor("xt", [C, T], bf16, kind="ExternalInput")
    wqkv_d = nc.dram_tensor("wqkv", [C, 768], bf16, kind="ExternalInput")
    bqk_d = nc.dram_tensor("bqk", [128, 4], f32, kind="ExternalInput")
    wproj_d = nc.dram_tensor("wproj", [256, C], bf16, kind="ExternalInput")
    out_d = nc.dram_tensor("out", [T, C], bf16, kind="ExternalOutput")

    with tile.TileContext(nc) as tc:
        with (
            tc.tile_pool(name="const", bufs=1) as const,
            tc.tile_pool(name="xts", bufs=3) as xts_pool,
            tc.tile_pool(name="pt", bufs=28) as pt_pool,
            tc.tile_pool(name="atts", bufs=8) as atts_pool,
            tc.tile_pool(name="rec", bufs=8) as rec_pool,
            tc.tile_pool(name="ot", bufs=8) as ot_pool,
            tc.tile_pool(name="ps_mm", bufs=2, space="PSUM") as ps_mm,
            tc.tile_pool(name="ps_sc", bufs=2, space="PSUM") as ps_sc,
            tc.tile_pool(name="ps_att", bufs=4, space="PSUM") as ps_att,
        ):
            # ---- resident tensors; DMAs chunked so compute starts early ----
            wqkv_sb = const.tile([128, NKC, 768], bf16, tag="wqkv")
            wqkv_r = wqkv_d.rearrange("(o p) n -> p o n", p=128)
            nc.scalar.dma_start(wqkv_sb[:, 0, :], wqkv_r[:, 0, :])
            nc.scalar.dma_start(wqkv_sb[:, 1:4, :], wqkv_r[:, 1:4, :])
            nc.scalar.dma_start(wqkv_sb[:, 4:NKC, :], wqkv_r[:, 4:NKC, :])
            bqk_sb = const.tile([128, 4], f32, tag="bqk")
            nc.scalar.dma_start(bqk_sb[:], bqk_d[:, :])
            wproj_sb = const.tile([128, 2, C], bf16, tag="wproj")
            nc.scalar.dma_start(wproj_sb[:], wproj_d.rearrange("(o p) n -> p o n", p=128))

            # qkT chunks: 0 = qT heads01, 1 = qT heads23, 2 = kT h01, 3 = kT h23
            qkT_sb = const.tile([128, 4, T], bf16, tag="qkT")
            # v in PV-rhs layout: [key mod 128, key block, head, 64 vdims + one]
            v_sb = const.tile([128, NTT, HPC, 65], bf16, tag="v")
            nc.vector.memset(v_sb[:, :, :, 64:65], 1.0)
            # attT: chunk hc: partitions = head-dims of heads (2hc, 2hc+1)
            attT_sb = const.tile([128, 2, T], bf16, tag="attT")
            # identity for PE-transpose of the normalized attention
            ident_sb = const.tile([128, 128], bf16, tag="ident")
            nc.vector.memset(ident_sb[:], 1.0)
            nc.gpsimd.affine_select(
                ident_sb[:],
                ident_sb[:],
                pattern=[[1, 128]],
                compare_op=ALU.is_equal,
                fill=0.0,
                base=0,
                channel_multiplier=-1,
            )
            # lower-triangular causal mask (keep j >= p), applied to diagonal
            # blocks with a DVE multiply (lower latency than gpsimd select)
            tri_sb = const.tile([128, 128], bf16, tag="tri")
            nc.vector.memset(tri_sb[:], 1.0)
            nc.gpsimd.affine_select(
                tri_sb[:],
                tri_sb[:],
                pattern=[[1, 128]],
                compare_op=ALU.is_ge,
                fill=0.0,
                base=0,
                channel_multiplier=-1,
            )

            xt_r = xt_d.rearrange("(o p) t -> p o t", p=128)

            def emit_qkv(tq):
                tqs = slice(TQ * tq, TQ * (tq + 1))
                xts = xts_pool.tile([128, NKC, TQ], bf16, tag="xts")
                nc.sync.dma_start(xts[:, 0, :], xt_r[:, 0, tqs])
                nc.sync.dma_start(xts[:, 1:4, :], xt_r[:, 1:4, tqs])
                nc.sync.dma_start(xts[:, 4:NKC, :], xt_r[:, 4:NKC, tqs])
                # q,k transposed: psum = wqkv_chunk.T @ xT
                if tq == 0:
                    # kc-major over pairs of open psum groups to hide DMA ramp
                    for cpp in range(2):
                        ps_pair = [
                            ps_mm.tile([128, TQ], f32, tag="mm", name=f"qk{cpp}{i}")
                            for i in range(2)
                        ]
                        for kc in range(NKC):
                            for i in range(2):
                                cp = 2 * cpp + i
                                nc.tensor.matmul(
                                    ps_pair[i][:],
                                    lhsT=wqkv_sb[:, kc, 128 * cp : 128 * (cp + 1)],
                                    rhs=xts[:, kc, :],
                                    start=(kc == 0),
                                    stop=(kc == NKC - 1),
                                    skip_group_check=True,
                                )
                        for i in range(2):
                            cp = 2 * cpp + i
                            nc.vector.tensor_scalar_add(
                                qkT_sb[:, cp, tqs], ps_pair[i][:], bqk_sb[:, cp : cp + 1]
                            )
                else:
                    for cp in range(4):
                        ps = ps_mm.tile([128, TQ], f32, tag="mm")
                        for kc in range(NKC):
                            nc.tensor.matmul(
                                ps[:],
                                lhsT=wqkv_sb[:, kc, 128 * cp : 128 * (cp + 1)],
                                rhs=xts[:, kc, :],
                                start=(kc == 0),
                                stop=(kc == NKC - 1),
                            )
                        nc.vector.tensor_scalar_add(
                            qkT_sb[:, cp, tqs], ps[:], bqk_sb[:, cp : cp + 1]
                        )
                # v: psum = xT_chunk.T @ wv  (no bias: folded into host output)
                for tt in range(4 * tq, 4 * tq + 4):
                    psv = ps_mm.tile([128, TQ], f32, tag="mm")
                    toff = 128 * tt - TQ * tq
                    for kc in range(NKC):
                        nc.tensor.matmul(
                            psv[:, 0:256],
                            lhsT=xts[:, kc, toff : toff + 128],
                            rhs=wqkv_sb[:, kc, 512:768],
                            start=(kc == 0),
                            stop=(kc == NKC - 1),
                        )
                    # [128, 256] psum -> [128, 4, 64] slot of v_sb (head-strided)
                    nc.vector.tensor_copy(v_sb[:, tt, :, 0:64], psv[:, 0:256])

            def qkv_units(tq):
                """qkv projection for t-slice tq as filler closures (one psum
                group each) interleaved into the attention instruction stream
                so the PE has independent work while Act runs exp."""
                tqs = slice(TQ * tq, TQ * (tq + 1))
                xts = xts_pool.tile([128, NKC, TQ], bf16, tag="xts")

                def load():
                    nc.sync.dma_start(xts[:, 0:4, :], xt_r[:, 0:4, tqs])
                    nc.sync.dma_start(xts[:, 4:NKC, :], xt_r[:, 4:NKC, tqs])

                def qk_unit(cp):
                    def emit():
                        ps = ps_mm.tile([128, TQ], f32, tag="mm")
                        for kc in range(NKC):
                            nc.tensor.matmul(
                                ps[:],
                                lhsT=wqkv_sb[:, kc, 128 * cp : 128 * (cp + 1)],
                                rhs=xts[:, kc, :],
                                start=(kc == 0),
                                stop=(kc == NKC - 1),
                            )
                        nc.vector.tensor_scalar_add(
                            qkT_sb[:, cp, tqs], ps[:], bqk_sb[:, cp : cp + 1]
                        )
                    return emit

                def v_unit(tt):
                    def emit():
                        psv = ps_mm.tile([128, TQ], f32, tag="mm")
                        toff = 128 * tt - TQ * tq
                        for kc in range(NKC):
                            nc.tensor.matmul(
                                psv[:, 0:256],
                                lhsT=xts[:, kc, toff : toff + 128],
                                rhs=wqkv_sb[:, kc, 512:768],
                                start=(kc == 0),
                                stop=(kc == NKC - 1),
                            )
                        nc.vector.tensor_copy(v_sb[:, tt, :, 0:64], psv[:, 0:256])
                    return emit

                return load, [qk_unit(cp) for cp in range(4)] + [
                    v_unit(tt) for tt in range(4 * tq, 4 * tq + 4)
                ]

            def proj_units_qb(tq, qb):
                tt = 4 * tq + qb
                ot = [None]

                def emit(nt):
                    ts_ = slice(128 * tt, 128 * (tt + 1))
                    ns = slice(512 * nt, 512 * (nt + 1))
                    pso = ps_mm.tile([128, TQ], f32, tag="mm")
                    for hc in range(2):
                        nc.tensor.matmul(
                            pso[:],
                            lhsT=attT_sb[:, hc, ts_],
                            rhs=wproj_sb[:, hc, ns],
                            start=(hc == 0),
                            stop=(hc == 1),
                        )
                    if nt == 0:
                        ot[0] = ot_pool.tile(
                            [128, 2, TQ], bf16, tag="ot", name=f"ot{tt}"
                        )
                    nc.vector.tensor_copy(ot[0][:, nt, :], pso[:])
                    if nt == 1:
                        # one merged DMA per 128-row block (fewer DMAs =
                        # less serialization on the single-slot HWDGE)
                        nc.sync.dma_start(out_d[ts_, :], ot[0][:])

                return [lambda: emit(0), lambda: emit(1)]

            def proj_units(tq):
                units = []
                for qb in range(4):
                    units.extend(proj_units_qb(tq, qb))
                return units

            def emit_att(tq, fillers, last=False):
                """Attention for tq with PV lagging scores by one step and
                filler matmul units spliced between, so the PE never idles on
                the exp (Act) latency. Each query block's normalize/transpose
                chain is emitted as soon as its diagonal block completes; on
                the last tq the projection units are appended to the filler
                queue the same way, collapsing the pipeline tail."""
                ntk = 4 * tq + 4
                attps = [
                    ps_att.tile([128, HPC, 65], f32, tag="att", name=f"att{tq}_{qb}")
                    for qb in range(4)
                ]
                steps = [(tk, h) for tk in range(ntk) for h in range(HPC)]
                pts = {}

                def emit_sc(i):
                    tk, h = steps[i]
                    d = tk - 4 * tq
                    q0 = 128 * d if d >= 0 else 0
                    w = TQ - q0
                    ks = slice(128 * tk, 128 * (tk + 1))
                    qs = slice(TQ * tq + q0, TQ * (tq + 1))
                    qc, kc_, pr = h // 2, 2 + h // 2, 64 * (h % 2)
                    sc = ps_sc.tile([128, TQ], f32, tag="sc")
                    nc.tensor.matmul(
                        sc[:, 0:w],
                        lhsT=qkT_sb[pr : pr + 64, kc_, ks],
                        rhs=qkT_sb[pr : pr + 64, qc, qs],
                    )
                    pt = pt_pool.tile([128, TQ], bf16, tag="pt")
                    nc.scalar.activation(pt[:, 0:w], sc[:, 0:w], AF.Exp, scale=SCALE)
                    if d >= 0:
                        nc.vector.tensor_mul(pt[:, 0:128], pt[:, 0:128], tri_sb[:])
                    pts[i] = pt

                def emit_pv(i):
                    tk, h = steps[i]
                    d = tk - 4 * tq
                    q0 = 128 * d if d >= 0 else 0
                    pt = pts.pop(i)
                    for qb in range(max(d, 0), 4):
                        qoff = 128 * qb - q0
                        nc.tensor.matmul(
                            attps[qb][:, h, 0:65],
                            lhsT=pt[:, qoff : qoff + 128],
                            rhs=v_sb[:, tk, h, 0:65],
                            start=(tk == 0 and h == 0),
                            stop=(tk == 4 * tq + qb and h == HPC - 1),
                            skip_group_check=True,
                        )

                def emit_norm(qb):
                    rec = rec_pool.tile([128, HPC], f32, tag="rec")
                    nc.vector.reciprocal_approx_fast(
                        out=rec[:], in_=attps[qb][:, :, 64]
                    )
                    att_sb = atts_pool.tile([128, HPC, 64], bf16, tag="atts")
                    nc.vector.tensor_mul(
                        att_sb[:],
                        attps[qb][:, :, 0:64],
                        rec[:, :, None].broadcast_to([128, HPC, 64]),
                    )
                    # transpose on the PE (att_sb [q, hd] -> attT [hd, q]):
                    # ~53ns each vs ~2.5us latency for the DMA XBAR route
                    qslice = slice(TQ * tq + 128 * qb, TQ * tq + 128 * (qb + 1))
                    attTps = ps_att.tile([128, 2, 128], bf16, tag="att")
                    for hc in range(2):
                        nc.tensor.matmul(
                            attTps[:, hc, :],
                            lhsT=att_sb[:, 2 * hc : 2 * hc + 2, :],
                            rhs=ident_sb[:],
                            is_transpose=True,
                            start=(hc == 0),
                            stop=(hc == 1),
                            skip_group_check=True,
                        )
                    nc.vector.tensor_copy(attT_sb[:, :, qslice], attTps[:])

                # PV lags scores by LAG steps so the exp(Act) + mask(DVE)
                # latency is hidden behind later scores/filler matmuls.
                # Dynamically appended fillers (last-tq proj units) are held
                # for DELAY steps: the normalize->transpose->proj readiness
                # chain is long, so scheduling them early just stalls the PE.
                # attention steps first (lowest priority index, so the
                # list scheduler never starves the Act engine), all filler
                # units after (they run whenever the PE would otherwise
                # stall, picked by readiness)
                LAG = 6
                ns = len(steps)
                dyn = []
                for i in range(ns + LAG):
                    if i < ns:
                        emit_sc(i)
                    j = i - LAG
                    if j >= 0:
                        emit_pv(j)
                        tk, h = steps[j]
                        if h == HPC - 1 and tk - 4 * tq >= 0:
                            qb = tk - 4 * tq
                            emit_norm(qb)
                            if last:
                                dyn.extend(proj_units_qb(tq, qb))
                for f in fillers:
                    f()
                for f in dyn:
                    f()

            # software pipeline: qkv(0) as a prologue; the per-tq attention
            # streams carry the remaining qkv/proj matmuls as fillers,
            # distributed by each attention block's Act-vs-PE deficit (the
            # later blocks are increasingly exp-bound, so all proj work is
            # pushed toward them; qkv(t) must complete before att(t) starts).
            emit_qkv(0)
            loads = {}
            plan = {0: [], 1: [], 2: [], 3: []}
            for t in (1, 2, 3):
                load, units = qkv_units(t)
                loads[t - 1] = load
                plan[t - 1] += units
            plan[3] += proj_units(0) + proj_units(1) + proj_units(2)
            for tq in range(NTQ):
                if tq in loads:
                    loads[tq]()
                emit_att(tq, plan[tq], last=(tq == NTQ - 1))

    nc.compile()
    return nc


def _shard_inputs(x, w_qkv, b_qkv, w_proj, b_proj):
    """Full inputs -> per-core input maps. Core c = (batch b=c//4, group g=c%4)."""
    in_maps = []
    xts = [np.ascontiguousarray(x[b].T).astype(ml_dtypes.bfloat16) for b in range(B)]
    for core in range(NCORES):
        b, g = divmod(core, 4)
        qs = slice(256 * g, 256 * (g + 1))
        ks = slice(C + 256 * g, C + 256 * (g + 1))
        vs = slice(2 * C + 256 * g, 2 * C + 256 * (g + 1))
        wqkv = np.concatenate(
            [w_qkv[:, qs], w_qkv[:, ks], w_qkv[:, vs]], axis=1
        ).astype(ml_dtypes.bfloat16)
        bq, bk = b_qkv[qs], b_qkv[ks]
        bqk = np.ascontiguousarray(
            np.stack([bq[0:128], bq[128:256], bk[0:128], bk[128:256]], axis=1)
        ).astype(np.float32)
        wproj = np.ascontiguousarray(w_proj[256 * g : 256 * (g + 1), :]).astype(
            ml_dtypes.bfloat16
        )
        in_maps.append(
            {"xt": xts[b], "wqkv": np.ascontiguousarray(wqkv), "bqk": bqk,
             "wproj": wproj}
        )
    return in_maps


def kernel(x, w_qkv, b_qkv, w_proj, b_proj):
    x = np.asarray(x, dtype=np.float32)
    w_qkv = np.asarray(w_qkv, dtype=np.float32)
    b_qkv = np.asarray(b_qkv, dtype=np.float32)
    w_proj = np.asarray(w_proj, dtype=np.float32)
    b_proj = np.asarray(b_proj, dtype=np.float32)

    if "nc" not in _CACHE:
        _CACHE["nc"] = build_nc()
    nc = _CACHE["nc"]

    in_maps = _shard_inputs(x, w_qkv, b_qkv, w_proj, b_proj)
    res = run_bass_kernel_spmd(nc, in_maps, list(range(NCORES)))
    # host epilogue: sum head-group partials, add folded bias
    b_eff = (b_qkv[2 * C :].astype(np.float64) @ w_proj.astype(np.float64)
             + b_proj).astype(np.float32)
    out = np.empty((B, T, C), dtype=np.float32)
    for b in range(B):
        acc = res.results[4 * b]["out"].astype(np.float32)
        for g in range(1, 4):
            acc = acc + res.results[4 * b + g]["out"].astype(np.float32)
        out[b] = acc + b_eff
    return out


# revision 9
# speedup vs baseline: 1.1549x; 1.0033x over previous
"""Multi-head causal self-attention (B=2, T=2048, C=1024, H=16, D=64) on 8
Trainium2 NeuronCores.

Sharding: data-parallel over batch (2) x tensor-parallel over heads (4 groups
of 4 heads) = 8 shards, no cross-core communication. Host sums the 4 partial
outputs per batch and adds the (folded) bias.

All matmuls in bf16 (1 PE cycle/row at any moving size; fp8/DoubleRow was
evaluated but every fp8 station exceeds the 2e-2 accuracy gate). Per core:
  qkT = wqk.T @ xT            [4x128, T]   (chunks: q01 q23 k01 k23)
  v   = xT.T @ wv             [T, 4, 64]+ones col (rhs layout for PV)
  per (tk 128-key block, head): scT = k_blk.T @ qT -> exp -> pt [keys, queries]
  PV non-transposed: att[q, h, 0:65] += pt[:, qblk].T @ v[:, tk, h, 0:65]
     (col 64 = ones -> per-query softmax denominator lands per PSUM partition,
      so normalization is one reciprocal + one broadcast multiply on the DVE)
  attT via PE transpose (identity matmul; the DMA XBAR route has ~2.5us
     chain latency that stalled the projection)
  out = attT.T @ wproj  -> bf16 out DMA; host adds b_proj + bv@wproj.

The attention inner loop is latency-bound (exp on the scalar engine), so the
qkv projection and output projection are split into filler units appended
AFTER each attention block's steps: the tile scheduler pops ready work by
emission-index priority, so attention (which feeds the saturated Act engine)
always wins ties while fillers absorb every PE stall. qkv(t) fills block
t-1; all projection work fills the final (most exp-bound) block. The causal
mask is a DVE multiply with a precomputed triangle; all bias matmuls are
folded into the host epilogue (softmax rows sum to 1, so the V bias
contributes bv @ w_proj to every output row).
"""

import numpy as np
import ml_dtypes

import concourse.bass as bass
import concourse.mybir as mybir
import concourse.tile as tile
from concourse import bacc
from concourse.bass_utils import run_bass_kernel_spmd

f32 = mybir.dt.float32
bf16 = mybir.dt.bfloat16
AF = mybir.ActivationFunctionType
ALU = mybir.AluOpType

B, T, C, H, D = 2, 2048, 1024, 16, 64
HPC = 4          # heads per core
NCORES = 8
TQ = 512         # query tile of the attention outer loop
NTQ = T // TQ    # 4
NKC = C // 128   # 8 contraction chunks for the qkv projection
NTT = T // 128   # 16 query 128-blocks
SCALE = 1.0 / 8.0  # 1/sqrt(D)

_CACHE = {}


def build_nc():
    nc = bacc.Bacc("TRN2", target_bir_lowering=False, debug=False)

    xt_d = nc.dram_tensor("xt", [C, T], bf16, kind="ExternalInput")
    wqkv_d = nc.dram_tensor("wqkv", [C, 768], bf16, kind="ExternalInput")
    bqk_d = nc.dram_tensor("bqk", [128, 4], f32, kind="ExternalInput")
    wproj_d = nc.dram_tensor("wproj", [256, C], bf16, kind="ExternalInput")
    out_d = nc.dram_tensor("out", [T, C], bf16, kind="ExternalOutput")

    with tile.TileContext(nc) as tc:
        with (
            tc.tile_pool(name="const", bufs=1) as const,
            tc.tile_pool(name="xts", bufs=3) as xts_pool,
            tc.tile_pool(name="pt", bufs=28) as pt_pool,
            tc.tile_pool(name="atts", bufs=8) as atts_pool,
            tc.tile_pool(name="rec", bufs=8) as rec_pool,
            tc.tile_pool(name="ot", bufs=8) as ot_pool,
            tc.tile_pool(name="ps_mm", bufs=2, space="PSUM") as ps_mm,
            tc.tile_pool(name="ps_sc", bufs=2, space="PSUM") as ps_sc,
            tc.tile_pool(name="ps_att", bufs=4, space="PSUM") as ps_att,
        ):
            # ---- resident tensors; DMAs chunked so compute starts early ----
            wqkv_sb = const.tile([128, NKC, 768], bf16, tag="wqkv")
            wqkv_r = wqkv_d.rearrange("(o p) n -> p o n", p=128)
            nc.scalar.dma_start(wqkv_sb[:, 0, :], wqkv_r[:, 0, :])
            nc.scalar.dma_start(wqkv_sb[:, 1:4, :], wqkv_r[:, 1:4, :])
            nc.scalar.dma_start(wqkv_sb[:, 4:NKC, :], wqkv_r[:, 4:NKC, :])
            bqk_sb = const.tile([128, 4], f32, tag="bqk")
            nc.scalar.dma_start(bqk_sb[:], bqk_d[:, :])
            wproj_sb = const.tile([128, 2, C], bf16, tag="wproj")
            nc.scalar.dma_start(wproj_sb[:], wproj_d.rearrange("(o p) n -> p o n", p=128))

            # qkT chunks: 0 = qT heads01, 1 = qT heads23, 2 = kT h01, 3 = kT h23
            qkT_sb = const.tile([128, 4, T], bf16, tag="qkT")
            # v in PV-rhs layout: [key mod 128, key block, head, 64 vdims + one]
            v_sb = const.tile([128, NTT, HPC, 65], bf16, tag="v")
            nc.vector.memset(v_sb[:, :, :, 64:65], 1.0)
            # attT: chunk hc: partitions = head-dims of heads (2hc, 2hc+1)
            attT_sb = const.tile([128, 2, T], bf16, tag="attT")
            # identity for PE-transpose of the normalized attention
            ident_sb = const.tile([128, 128], bf16, tag="ident")
            nc.vector.memset(ident_sb[:], 1.0)
            nc.gpsimd.affine_select(
                ident_sb[:],
                ident_sb[:],
                pattern=[[1, 128]],
                compare_op=ALU.is_equal,
                fill=0.0,
                base=0,
                channel_multiplier=-1,
            )
            # lower-triangular causal mask (keep j >= p), applied to diagonal
            # blocks with a DVE multiply (lower latency than gpsimd select)
            tri_sb = const.tile([128, 128], bf16, tag="tri")
            nc.vector.memset(tri_sb[:], 1.0)
            nc.gpsimd.affine_select(
                tri_sb[:],
                tri_sb[:],
                pattern=[[1, 128]],
                compare_op=ALU.is_ge,
                fill=0.0,
                base=0,
                channel_multiplier=-1,
            )

            xt_r = xt_d.rearrange("(o p) t -> p o t", p=128)

            def emit_qkv(tq):
                tqs = slice(TQ * tq, TQ * (tq + 1))
                xts = xts_pool.tile([128, NKC, TQ], bf16, tag="xts")
                nc.sync.dma_start(xts[:, 0, :], xt_r[:, 0, tqs])
                nc.sync.dma_start(xts[:, 1:4, :], xt_r[:, 1:4, tqs])
                nc.sync.dma_start(xts[:, 4:NKC, :], xt_r[:, 4:NKC, tqs])
                # q,k transposed: psum = wqkv_chunk.T @ xT
                if tq == 0:
                    # kc-major over pairs of open psum groups to hide DMA ramp
                    for cpp in range(2):
                        ps_pair = [
                            ps_mm.tile([128, TQ], f32, tag="mm", name=f"qk{cpp}{i}")
                            for i in range(2)
                        ]
                        for kc in range(NKC):
                            for i in range(2):
                                cp = 2 * cpp + i
                                nc.tensor.matmul(
                                    ps_pair[i][:],
                                    lhsT=wqkv_sb[:, kc, 128 * cp : 128 * (cp + 1)],
                                    rhs=xts[:, kc, :],
                                    start=(kc == 0),
                                    stop=(kc == NKC - 1),
                                    skip_group_check=True,
                                )
                        for i in range(2):
                            cp = 2 * cpp + i
                            nc.vector.tensor_scalar_add(
                                qkT_sb[:, cp, tqs], ps_pair[i][:], bqk_sb[:, cp : cp + 1]
                            )
                else:
                    for cp in range(4):
                        ps = ps_mm.tile([128, TQ], f32, tag="mm")
                        for kc in range(NKC):
                            nc.tensor.matmul(
                                ps[:],
                                lhsT=wqkv_sb[:, kc, 128 * cp : 128 * (cp + 1)],
                                rhs=xts[:, kc, :],
                                start=(kc == 0),
                                stop=(kc == NKC - 1),
                            )
                        nc.vector.tensor_scalar_add(
                            qkT_sb[:, cp, tqs], ps[:], bqk_sb[:, cp : cp + 1]
                        )
                # v: psum = xT_chunk.T @ wv  (no bias: folded into host output)
                for tt in range(4 * tq, 4 * tq + 4):
                    psv = ps_mm.tile([128, TQ], f32, tag="mm")
                    toff = 128 * tt - TQ * tq
                    for kc in range(NKC):
                        nc.tensor.matmul(
                            psv[:, 0:256],
                            lhsT=xts[:, kc, toff : toff + 128],
                            rhs=wqkv_sb[:, kc, 512:768],
                            start=(kc == 0),
                            stop=(kc == NKC - 1),
                        )
                    # [128, 256] psum -> [128, 4, 64] slot of v_sb (head-strided)
                    nc.vector.tensor_copy(v_sb[:, tt, :, 0:64], psv[:, 0:256])

            def qkv_units(tq):
                """qkv projection for t-slice tq as filler closures (one psum
                group each) interleaved into the attention instruction stream
                so the PE has independent work while Act runs exp."""
                tqs = slice(TQ * tq, TQ * (tq + 1))
                xts = xts_pool.tile([128, NKC, TQ], bf16, tag="xts")

                def load():
                    nc.sync.dma_start(xts[:, 0:4, :], xt_r[:, 0:4, tqs])
                    nc.sync.dma_start(xts[:, 4:NKC, :], xt_r[:, 4:NKC, tqs])

                def qk_unit(cp):
                    def emit():
                        ps = ps_mm.tile([128, TQ], f32, tag="mm")
                        for kc in range(NKC):
                            nc.tensor.matmul(
                                ps[:],
                                lhsT=wqkv_sb[:, kc, 128 * cp : 128 * (cp + 1)],
                                rhs=xts[:, kc, :],
                                start=(kc == 0),
                                stop=(kc == NKC - 1),
                            )
                        nc.vector.tensor_scalar_add(
                            qkT_sb[:, cp, tqs], ps[:], bqk_sb[:, cp : cp + 1]
                        )
                    return emit

                def v_unit(tt):
                    def emit():
                        psv = ps_mm.tile([128, TQ], f32, tag="mm")
                        toff = 128 * tt - TQ * tq
                        for kc in range(NKC):
                            nc.tensor.matmul(
                                psv[:, 0:256],
                                lhsT=xts[:, kc, toff : toff + 128],
                                rhs=wqkv_sb[:, kc, 512:768],
                                start=(kc == 0),
                                stop=(kc == NKC - 1),
                            )
                        nc.vector.tensor_copy(v_sb[:, tt, :, 0:64], psv[:, 0:256])
                    return emit

                return load, [qk_unit(cp) for cp in range(4)] + [
                    v_unit(tt) for tt in range(4 * tq, 4 * tq + 4)
                ]

            def proj_units_qb(tq, qb):
                tt = 4 * tq + qb
                ot = [None]

                def emit(nt):
                    ts_ = slice(128 * tt, 128 * (tt + 1))
                    ns = slice(512 * nt, 512 * (nt + 1))
                    pso = ps_mm.tile([128, TQ], f32, tag="mm")
                    for hc in range(2):
                        nc.tensor.matmul(
                            pso[:],
                            lhsT=attT_sb[:, hc, ts_],
                            rhs=wproj_sb[:, hc, ns],
                            start=(hc == 0),
                            stop=(hc == 1),
                        )
                    if nt == 0:
                        ot[0] = ot_pool.tile(
                            [128, 2, TQ], bf16, tag="ot", name=f"ot{tt}"
                        )
                    if tq == NTQ - 1:
                        # last block: Act is done with exps by now while the
                        # DVE still drains normalize chains - use Act for the
                        # tail's psum copies
                        nc.scalar.copy(ot[0][:, nt, :], pso[:])
                    else:
                        nc.vector.tensor_copy(ot[0][:, nt, :], pso[:])
                    if nt == 1:
                        # one merged DMA per 128-row block (fewer DMAs =
                        # less serialization on the single-slot HWDGE)
                        nc.sync.dma_start(out_d[ts_, :], ot[0][:])

                return [lambda: emit(0), lambda: emit(1)]

            def proj_units(tq):
                units = []
                for qb in range(4):
                    units.extend(proj_units_qb(tq, qb))
                return units

            def emit_att(tq, fillers, last=False):
                """Attention for tq with PV lagging scores by one step and
                filler matmul units spliced between, so the PE never idles on
                the exp (Act) latency. Each query block's normalize/transpose
                chain is emitted as soon as its diagonal block completes; on
                the last tq the projection units are appended to the filler
                queue the same way, collapsing the pipeline tail."""
                ntk = 4 * tq + 4
                attps = [
                    ps_att.tile([128, HPC, 65], f32, tag="att", name=f"att{tq}_{qb}")
                    for qb in range(4)
                ]
                steps = [(tk, h) for tk in range(ntk) for h in range(HPC)]
                pts = {}

                def emit_sc(i):
                    tk, h = steps[i]
                    d = tk - 4 * tq
                    q0 = 128 * d if d >= 0 else 0
                    w = TQ - q0
                    ks = slice(128 * tk, 128 * (tk + 1))
                    qs = slice(TQ * tq + q0, TQ * (tq + 1))
                    qc, kc_, pr = h // 2, 2 + h // 2, 64 * (h % 2)
                    sc = ps_sc.tile([128, TQ], f32, tag="sc")
                    nc.tensor.matmul(
                        sc[:, 0:w],
                        lhsT=qkT_sb[pr : pr + 64, kc_, ks],
                        rhs=qkT_sb[pr : pr + 64, qc, qs],
                    )
                    pt = pt_pool.tile([128, TQ], bf16, tag="pt")
                    nc.scalar.activation(pt[:, 0:w], sc[:, 0:w], AF.Exp, scale=SCALE)
                    if d >= 0:
                        nc.vector.tensor_mul(pt[:, 0:128], pt[:, 0:128], tri_sb[:])
                    pts[i] = pt

                def emit_pv(i):
                    tk, h = steps[i]
                    d = tk - 4 * tq
                    q0 = 128 * d if d >= 0 else 0
                    pt = pts.pop(i)
                    for qb in range(max(d, 0), 4):
                        qoff = 128 * qb - q0
                        nc.tensor.matmul(
                            attps[qb][:, h, 0:65],
                            lhsT=pt[:, qoff : qoff + 128],
                            rhs=v_sb[:, tk, h, 0:65],
                            start=(tk == 0 and h == 0),
                            stop=(tk == 4 * tq + qb and h == HPC - 1),
                            skip_group_check=True,
                        )

                def emit_norm(qb):
                    rec = rec_pool.tile([128, HPC], f32, tag="rec")
                    nc.vector.reciprocal_approx_fast(
                        out=rec[:], in_=attps[qb][:, :, 64]
                    )
                    att_sb = atts_pool.tile([128, HPC, 64], bf16, tag="atts")
                    nc.vector.tensor_mul(
                        att_sb[:],
                        attps[qb][:, :, 0:64],
                        rec[:, :, None].broadcast_to([128, HPC, 64]),
                    )
                    # transpose on the PE (att_sb [q, hd] -> attT [hd, q]):
                    # ~53ns each vs ~2.5us latency for the DMA XBAR route
                    qslice = slice(TQ * tq + 128 * qb, TQ * tq + 128 * (qb + 1))
                    attTps = ps_att.tile([128, 2, 128], bf16, tag="att")
                    for hc in range(2):
                        nc.tensor.matmul(
                            attTps[:, hc, :],
                            lhsT=att_sb[:, 2 * hc : 2 * hc + 2, :],
                            rhs=ident_sb[:],
                            is_transpose=True,
                            start=(hc == 0),
                            stop=(hc == 1),
                            skip_group_check=True,
                        )
                    nc.vector.tensor_copy(attT_sb[:, :, qslice], attTps[:])

                # PV lags scores by LAG steps so the exp(Act) + mask(DVE)
                # latency is hidden behind later scores/filler matmuls.
                # Dynamically appended fillers (last-tq proj units) are held
                # for DELAY steps: the normalize->transpose->proj readiness
                # chain is long, so scheduling them early just stalls the PE.
                # attention steps first (lowest priority index, so the
                # list scheduler never starves the Act engine), all filler
                # units after (they run whenever the PE would otherwise
                # stall, picked by readiness)
                LAG = 6
                ns = len(steps)
                dyn = []
                for i in range(ns + LAG):
                    if i < ns:
                        emit_sc(i)
                    j = i - LAG
                    if j >= 0:
                        emit_pv(j)
                        tk, h = steps[j]
                        if h == HPC - 1 and tk - 4 * tq >= 0:
                            qb = tk - 4 * tq
                            emit_norm(qb)
                            if last:
                                dyn.extend(proj_units_qb(tq, qb))
                for f in fillers:
                    f()
                for f in dyn:
                    f()

            # software pipeline: qkv(0) as a prologue; the per-tq attention
            # streams carry the remaining qkv/proj matmuls as fillers,
            # distributed by each attention block's Act-vs-PE deficit (the
            # later blocks are increasingly exp-bound, so all proj work is
            # pushed toward them; qkv(t) must complete before att(t) starts).
            emit_qkv(0)
            loads = {}
            plan = {0: [], 1: [], 2: [], 3: []}
            for t in (1, 2, 3):
                load, units = qkv_units(t)
                loads[t - 1] = load
                plan[t - 1] += units
            plan[3] += proj_units(0) + proj_units(1) + proj_units(2)
            for tq in range(NTQ):
                if tq in loads:
                    loads[tq]()
                emit_att(tq, plan[tq], last=(tq == NTQ - 1))

    nc.compile()
    return nc


def _shard_inputs(x, w_qkv, b_qkv, w_proj, b_proj):
    """Full inputs -> per-core input maps. Core c = (batch b=c//4, group g=c%4)."""
    in_maps = []
    xts = [np.ascontiguousarray(x[b].T).astype(ml_dtypes.bfloat16) for b in range(B)]
    for core in range(NCORES):
        b, g = divmod(core, 4)
        qs = slice(256 * g, 256 * (g + 1))
        ks = slice(C + 256 * g, C + 256 * (g + 1))
        vs = slice(2 * C + 256 * g, 2 * C + 256 * (g + 1))
        wqkv = np.concatenate(
            [w_qkv[:, qs], w_qkv[:, ks], w_qkv[:, vs]], axis=1
        ).astype(ml_dtypes.bfloat16)
        bq, bk = b_qkv[qs], b_qkv[ks]
        bqk = np.ascontiguousarray(
            np.stack([bq[0:128], bq[128:256], bk[0:128], bk[128:256]], axis=1)
        ).astype(np.float32)
        wproj = np.ascontiguousarray(w_proj[256 * g : 256 * (g + 1), :]).astype(
            ml_dtypes.bfloat16
        )
        in_maps.append(
            {"xt": xts[b], "wqkv": np.ascontiguousarray(wqkv), "bqk": bqk,
             "wproj": wproj}
        )
    return in_maps


def kernel(x, w_qkv, b_qkv, w_proj, b_proj):
    x = np.asarray(x, dtype=np.float32)
    w_qkv = np.asarray(w_qkv, dtype=np.float32)
    b_qkv = np.asarray(b_qkv, dtype=np.float32)
    w_proj = np.asarray(w_proj, dtype=np.float32)
    b_proj = np.asarray(b_proj, dtype=np.float32)

    if "nc" not in _CACHE:
        _CACHE["nc"] = build_nc()
    nc = _CACHE["nc"]

    in_maps = _shard_inputs(x, w_qkv, b_qkv, w_proj, b_proj)
    res = run_bass_kernel_spmd(nc, in_maps, list(range(NCORES)))
    # host epilogue: sum head-group partials, add folded bias
    b_eff = (b_qkv[2 * C :].astype(np.float64) @ w_proj.astype(np.float64)
             + b_proj).astype(np.float32)
    out = np.empty((B, T, C), dtype=np.float32)
    for b in range(B):
        acc = res.results[4 * b]["out"].astype(np.float32)
        for g in range(1, 4):
            acc = acc + res.results[4 * b + g]["out"].astype(np.float32)
        out[b] = acc + b_eff
    return out


# revision 10
# speedup vs baseline: 1.1565x; 1.0014x over previous
"""Multi-head causal self-attention (B=2, T=2048, C=1024, H=16, D=64) on 8
Trainium2 NeuronCores.

Sharding: data-parallel over batch (2) x tensor-parallel over heads (4 groups
of 4 heads) = 8 shards, no cross-core communication. Host sums the 4 partial
outputs per batch and adds the (folded) bias.

All matmuls in bf16 (1 PE cycle/row at any moving size; fp8/DoubleRow was
evaluated but every fp8 station exceeds the 2e-2 accuracy gate). Per core:
  qkT = wqk.T @ xT            [4x128, T]   (chunks: q01 q23 k01 k23)
  v   = xT.T @ wv             [T, 4, 64]+ones col (rhs layout for PV)
  per (tk 128-key block, head): scT = k_blk.T @ qT -> exp -> pt [keys, queries]
  PV non-transposed: att[q, h, 0:65] += pt[:, qblk].T @ v[:, tk, h, 0:65]
     (col 64 = ones -> per-query softmax denominator lands per PSUM partition,
      so normalization is one reciprocal + one broadcast multiply on the DVE)
  attT via PE transpose (identity matmul; the DMA XBAR route has ~2.5us
     chain latency that stalled the projection)
  out = attT.T @ wproj  -> bf16 out DMA; host adds b_proj + bv@wproj.

The attention inner loop is latency-bound (exp on the scalar engine), so the
qkv projection and output projection are split into filler units appended
AFTER each attention block's steps: the tile scheduler pops ready work by
emission-index priority, so attention (which feeds the saturated Act engine)
always wins ties while fillers absorb every PE stall. qkv(t) fills block
t-1; all projection work fills the final (most exp-bound) block. The causal
mask is a DVE multiply with a precomputed triangle; all bias matmuls are
folded into the host epilogue (softmax rows sum to 1, so the V bias
contributes bv @ w_proj to every output row).
"""

import numpy as np
import ml_dtypes

import concourse.bass as bass
import concourse.mybir as mybir
import concourse.tile as tile
from concourse import bacc
from concourse.bass_utils import run_bass_kernel_spmd

f32 = mybir.dt.float32
bf16 = mybir.dt.bfloat16
AF = mybir.ActivationFunctionType
ALU = mybir.AluOpType

B, T, C, H, D = 2, 2048, 1024, 16, 64
HPC = 4          # heads per core
NCORES = 8
TQ = 512         # query tile of the attention outer loop
NTQ = T // TQ    # 4
NKC = C // 128   # 8 contraction chunks for the qkv projection
NTT = T // 128   # 16 query 128-blocks
SCALE = 1.0 / 8.0  # 1/sqrt(D)

_CACHE = {}


def build_nc():
    nc = bacc.Bacc("TRN2", target_bir_lowering=False, debug=False)

    xt_d = nc.dram_tensor("xt", [C, T], bf16, kind="ExternalInput")
    wqkv_d = nc.dram_tensor("wqkv", [C, 768], bf16, kind="ExternalInput")
    bqk_d = nc.dram_tensor("bqk", [128, 4], f32, kind="ExternalInput")
    wproj_d = nc.dram_tensor("wproj", [256, C], bf16, kind="ExternalInput")
    out_d = nc.dram_tensor("out", [T, C], bf16, kind="ExternalOutput")

    with tile.TileContext(nc) as tc:
        with (
            tc.tile_pool(name="const", bufs=1) as const,
            tc.tile_pool(name="xts", bufs=3) as xts_pool,
            tc.tile_pool(name="pt", bufs=28) as pt_pool,
            tc.tile_pool(name="atts", bufs=8) as atts_pool,
            tc.tile_pool(name="rec", bufs=8) as rec_pool,
            tc.tile_pool(name="ot", bufs=8) as ot_pool,
            tc.tile_pool(name="ps_mm", bufs=2, space="PSUM") as ps_mm,
            tc.tile_pool(name="ps_sc", bufs=2, space="PSUM") as ps_sc,
            tc.tile_pool(name="ps_att", bufs=4, space="PSUM") as ps_att,
        ):
            # ---- resident tensors; DMAs chunked so compute starts early ----
            wqkv_sb = const.tile([128, NKC, 768], bf16, tag="wqkv")
            wqkv_r = wqkv_d.rearrange("(o p) n -> p o n", p=128)
            nc.scalar.dma_start(wqkv_sb[:, 0, :], wqkv_r[:, 0, :])
            nc.scalar.dma_start(wqkv_sb[:, 1:4, :], wqkv_r[:, 1:4, :])
            nc.scalar.dma_start(wqkv_sb[:, 4:NKC, :], wqkv_r[:, 4:NKC, :])
            bqk_sb = const.tile([128, 4], f32, tag="bqk")
            nc.scalar.dma_start(bqk_sb[:], bqk_d[:, :])
            wproj_sb = const.tile([128, 2, C], bf16, tag="wproj")
            nc.scalar.dma_start(wproj_sb[:], wproj_d.rearrange("(o p) n -> p o n", p=128))

            # qkT chunks: 0 = qT heads01, 1 = qT heads23, 2 = kT h01, 3 = kT h23
            qkT_sb = const.tile([128, 4, T], bf16, tag="qkT")
            # v in PV-rhs layout: [key mod 128, key block, head, 64 vdims + one]
            v_sb = const.tile([128, NTT, HPC, 65], bf16, tag="v")
            nc.vector.memset(v_sb[:, :, :, 64:65], 1.0)
            # attT: chunk hc: partitions = head-dims of heads (2hc, 2hc+1)
            attT_sb = const.tile([128, 2, T], bf16, tag="attT")
            # identity for PE-transpose of the normalized attention
            ident_sb = const.tile([128, 128], bf16, tag="ident")
            nc.vector.memset(ident_sb[:], 1.0)
            nc.gpsimd.affine_select(
                ident_sb[:],
                ident_sb[:],
                pattern=[[1, 128]],
                compare_op=ALU.is_equal,
                fill=0.0,
                base=0,
                channel_multiplier=-1,
            )
            # lower-triangular causal mask (keep j >= p), applied to diagonal
            # blocks with a DVE multiply (lower latency than gpsimd select)
            tri_sb = const.tile([128, 128], bf16, tag="tri")
            nc.vector.memset(tri_sb[:], 1.0)
            nc.gpsimd.affine_select(
                tri_sb[:],
                tri_sb[:],
                pattern=[[1, 128]],
                compare_op=ALU.is_ge,
                fill=0.0,
                base=0,
                channel_multiplier=-1,
            )

            xt_r = xt_d.rearrange("(o p) t -> p o t", p=128)

            def emit_qkv(tq):
                tqs = slice(TQ * tq, TQ * (tq + 1))
                xts = xts_pool.tile([128, NKC, TQ], bf16, tag="xts")
                nc.sync.dma_start(xts[:, 0, :], xt_r[:, 0, tqs])
                nc.sync.dma_start(xts[:, 1:4, :], xt_r[:, 1:4, tqs])
                nc.sync.dma_start(xts[:, 4:NKC, :], xt_r[:, 4:NKC, tqs])
                # q,k transposed: psum = wqkv_chunk.T @ xT
                if tq == 0:
                    # kc-major over pairs of open psum groups to hide DMA ramp
                    for cps in ((0, 2), (1, 3)):
                        ps_pair = [
                            ps_mm.tile([128, TQ], f32, tag="mm", name=f"qk{cp}")
                            for cp in cps
                        ]
                        for kc in range(NKC):
                            for i, cp in enumerate(cps):
                                nc.tensor.matmul(
                                    ps_pair[i][:],
                                    lhsT=wqkv_sb[:, kc, 128 * cp : 128 * (cp + 1)],
                                    rhs=xts[:, kc, :],
                                    start=(kc == 0),
                                    stop=(kc == NKC - 1),
                                    skip_group_check=True,
                                )
                        for i, cp in enumerate(cps):
                            nc.vector.tensor_scalar_add(
                                qkT_sb[:, cp, tqs], ps_pair[i][:], bqk_sb[:, cp : cp + 1]
                            )
                else:
                    for cp in range(4):
                        ps = ps_mm.tile([128, TQ], f32, tag="mm")
                        for kc in range(NKC):
                            nc.tensor.matmul(
                                ps[:],
                                lhsT=wqkv_sb[:, kc, 128 * cp : 128 * (cp + 1)],
                                rhs=xts[:, kc, :],
                                start=(kc == 0),
                                stop=(kc == NKC - 1),
                            )
                        nc.vector.tensor_scalar_add(
                            qkT_sb[:, cp, tqs], ps[:], bqk_sb[:, cp : cp + 1]
                        )
                # v: psum = xT_chunk.T @ wv  (no bias: folded into host output)
                for tt in range(4 * tq, 4 * tq + 4):
                    psv = ps_mm.tile([128, TQ], f32, tag="mm")
                    toff = 128 * tt - TQ * tq
                    for kc in range(NKC):
                        nc.tensor.matmul(
                            psv[:, 0:256],
                            lhsT=xts[:, kc, toff : toff + 128],
                            rhs=wqkv_sb[:, kc, 512:768],
                            start=(kc == 0),
                            stop=(kc == NKC - 1),
                        )
                    # [128, 256] psum -> [128, 4, 64] slot of v_sb (head-strided)
                    nc.vector.tensor_copy(v_sb[:, tt, :, 0:64], psv[:, 0:256])

            def qkv_units(tq):
                """qkv projection for t-slice tq as filler closures (one psum
                group each) interleaved into the attention instruction stream
                so the PE has independent work while Act runs exp."""
                tqs = slice(TQ * tq, TQ * (tq + 1))
                xts = xts_pool.tile([128, NKC, TQ], bf16, tag="xts")

                def load():
                    nc.sync.dma_start(xts[:, 0:4, :], xt_r[:, 0:4, tqs])
                    nc.sync.dma_start(xts[:, 4:NKC, :], xt_r[:, 4:NKC, tqs])

                def qk_unit(cp):
                    def emit():
                        ps = ps_mm.tile([128, TQ], f32, tag="mm")
                        for kc in range(NKC):
                            nc.tensor.matmul(
                                ps[:],
                                lhsT=wqkv_sb[:, kc, 128 * cp : 128 * (cp + 1)],
                                rhs=xts[:, kc, :],
                                start=(kc == 0),
                                stop=(kc == NKC - 1),
                            )
                        nc.vector.tensor_scalar_add(
                            qkT_sb[:, cp, tqs], ps[:], bqk_sb[:, cp : cp + 1]
                        )
                    return emit

                def v_unit(tt):
                    def emit():
                        psv = ps_mm.tile([128, TQ], f32, tag="mm")
                        toff = 128 * tt - TQ * tq
                        for kc in range(NKC):
                            nc.tensor.matmul(
                                psv[:, 0:256],
                                lhsT=xts[:, kc, toff : toff + 128],
                                rhs=wqkv_sb[:, kc, 512:768],
                                start=(kc == 0),
                                stop=(kc == NKC - 1),
                            )
                        nc.vector.tensor_copy(v_sb[:, tt, :, 0:64], psv[:, 0:256])
                    return emit

                return load, [qk_unit(cp) for cp in range(4)] + [
                    v_unit(tt) for tt in range(4 * tq, 4 * tq + 4)
                ]

            def proj_units_qb(tq, qb):
                tt = 4 * tq + qb
                ot = [None]

                def emit(nt):
                    ts_ = slice(128 * tt, 128 * (tt + 1))
                    ns = slice(512 * nt, 512 * (nt + 1))
                    pso = ps_mm.tile([128, TQ], f32, tag="mm")
                    for hc in range(2):
                        nc.tensor.matmul(
                            pso[:],
                            lhsT=attT_sb[:, hc, ts_],
                            rhs=wproj_sb[:, hc, ns],
                            start=(hc == 0),
                            stop=(hc == 1),
                        )
                    if nt == 0:
                        ot[0] = ot_pool.tile(
                            [128, 2, TQ], bf16, tag="ot", name=f"ot{tt}"
                        )
                    if tq == NTQ - 1:
                        # last block: Act is done with exps by now while the
                        # DVE still drains normalize chains - use Act for the
                        # tail's psum copies
                        nc.scalar.copy(ot[0][:, nt, :], pso[:])
                    else:
                        nc.vector.tensor_copy(ot[0][:, nt, :], pso[:])
                    if nt == 1:
                        # one merged DMA per 128-row block (fewer DMAs =
                        # less serialization on the single-slot HWDGE)
                        nc.sync.dma_start(out_d[ts_, :], ot[0][:])

                return [lambda: emit(0), lambda: emit(1)]

            def proj_units(tq):
                units = []
                for qb in range(4):
                    units.extend(proj_units_qb(tq, qb))
                return units

            def emit_att(tq, fillers, last=False):
                """Attention for tq with PV lagging scores by one step and
                filler matmul units spliced between, so the PE never idles on
                the exp (Act) latency. Each query block's normalize/transpose
                chain is emitted as soon as its diagonal block completes; on
                the last tq the projection units are appended to the filler
                queue the same way, collapsing the pipeline tail."""
                ntk = 4 * tq + 4
                attps = [
                    ps_att.tile([128, HPC, 65], f32, tag="att", name=f"att{tq}_{qb}")
                    for qb in range(4)
                ]
                if tq == 0:
                    steps = [(tk, h) for hp in range(2) for tk in range(ntk)
                             for h in (2 * hp, 2 * hp + 1)]
                else:
                    steps = [(tk, h) for tk in range(ntk) for h in range(HPC)]
                pts = {}

                def emit_sc(i):
                    tk, h = steps[i]
                    d = tk - 4 * tq
                    q0 = 128 * d if d >= 0 else 0
                    w = TQ - q0
                    ks = slice(128 * tk, 128 * (tk + 1))
                    qs = slice(TQ * tq + q0, TQ * (tq + 1))
                    qc, kc_, pr = h // 2, 2 + h // 2, 64 * (h % 2)
                    sc = ps_sc.tile([128, TQ], f32, tag="sc")
                    nc.tensor.matmul(
                        sc[:, 0:w],
                        lhsT=qkT_sb[pr : pr + 64, kc_, ks],
                        rhs=qkT_sb[pr : pr + 64, qc, qs],
                    )
                    pt = pt_pool.tile([128, TQ], bf16, tag="pt")
                    nc.scalar.activation(pt[:, 0:w], sc[:, 0:w], AF.Exp, scale=SCALE)
                    if d >= 0:
                        nc.vector.tensor_mul(pt[:, 0:128], pt[:, 0:128], tri_sb[:])
                    pts[i] = pt

                def emit_pv(i):
                    tk, h = steps[i]
                    d = tk - 4 * tq
                    q0 = 128 * d if d >= 0 else 0
                    pt = pts.pop(i)
                    for qb in range(max(d, 0), 4):
                        qoff = 128 * qb - q0
                        nc.tensor.matmul(
                            attps[qb][:, h, 0:65],
                            lhsT=pt[:, qoff : qoff + 128],
                            rhs=v_sb[:, tk, h, 0:65],
                            start=(tk == 0 and h == 0),
                            stop=(tk == 4 * tq + qb and h == HPC - 1),
                            skip_group_check=True,
                        )

                def emit_norm(qb):
                    rec = rec_pool.tile([128, HPC], f32, tag="rec")
                    nc.vector.reciprocal_approx_fast(
                        out=rec[:], in_=attps[qb][:, :, 64]
                    )
                    att_sb = atts_pool.tile([128, HPC, 64], bf16, tag="atts")
                    nc.vector.tensor_mul(
                        att_sb[:],
                        attps[qb][:, :, 0:64],
                        rec[:, :, None].broadcast_to([128, HPC, 64]),
                    )
                    # transpose on the PE (att_sb [q, hd] -> attT [hd, q]):
                    # ~53ns each vs ~2.5us latency for the DMA XBAR route
                    qslice = slice(TQ * tq + 128 * qb, TQ * tq + 128 * (qb + 1))
                    attTps = ps_att.tile([128, 2, 128], bf16, tag="att")
                    for hc in range(2):
                        nc.tensor.matmul(
                            attTps[:, hc, :],
                            lhsT=att_sb[:, 2 * hc : 2 * hc + 2, :],
                            rhs=ident_sb[:],
                            is_transpose=True,
                            start=(hc == 0),
                            stop=(hc == 1),
                            skip_group_check=True,
                        )
                    nc.vector.tensor_copy(attT_sb[:, :, qslice], attTps[:])

                # PV lags scores by LAG steps so the exp(Act) + mask(DVE)
                # latency is hidden behind later scores/filler matmuls.
                # Dynamically appended fillers (last-tq proj units) are held
                # for DELAY steps: the normalize->transpose->proj readiness
                # chain is long, so scheduling them early just stalls the PE.
                # attention steps first (lowest priority index, so the
                # list scheduler never starves the Act engine), all filler
                # units after (they run whenever the PE would otherwise
                # stall, picked by readiness)
                LAG = 6
                ns = len(steps)
                dyn = []
                for i in range(ns + LAG):
                    if i < ns:
                        emit_sc(i)
                    j = i - LAG
                    if j >= 0:
                        emit_pv(j)
                        tk, h = steps[j]
                        if h == HPC - 1 and tk - 4 * tq >= 0:
                            qb = tk - 4 * tq
                            emit_norm(qb)
                            if last:
                                dyn.extend(proj_units_qb(tq, qb))
                for f in fillers:
                    f()
                for f in dyn:
                    f()

            # software pipeline: qkv(0) as a prologue; the per-tq attention
            # streams carry the remaining qkv/proj matmuls as fillers,
            # distributed by each attention block's Act-vs-PE deficit (the
            # later blocks are increasingly exp-bound, so all proj work is
            # pushed toward them; qkv(t) must complete before att(t) starts).
            emit_qkv(0)
            loads = {}
            plan = {0: [], 1: [], 2: [], 3: []}
            for t in (1, 2, 3):
                load, units = qkv_units(t)
                loads[t - 1] = load
                plan[t - 1] += units
            plan[3] += proj_units(0) + proj_units(1) + proj_units(2)
            for tq in range(NTQ):
                if tq in loads:
                    loads[tq]()
                emit_att(tq, plan[tq], last=(tq == NTQ - 1))

    nc.compile()
    return nc


def _shard_inputs(x, w_qkv, b_qkv, w_proj, b_proj):
    """Full inputs -> per-core input maps. Core c = (batch b=c//4, group g=c%4)."""
    in_maps = []
    xts = [np.ascontiguousarray(x[b].T).astype(ml_dtypes.bfloat16) for b in range(B)]
    for core in range(NCORES):
        b, g = divmod(core, 4)
        qs = slice(256 * g, 256 * (g + 1))
        ks = slice(C + 256 * g, C + 256 * (g + 1))
        vs = slice(2 * C + 256 * g, 2 * C + 256 * (g + 1))
        wqkv = np.concatenate(
            [w_qkv[:, qs], w_qkv[:, ks], w_qkv[:, vs]], axis=1
        ).astype(ml_dtypes.bfloat16)
        bq, bk = b_qkv[qs], b_qkv[ks]
        bqk = np.ascontiguousarray(
            np.stack([bq[0:128], bq[128:256], bk[0:128], bk[128:256]], axis=1)
        ).astype(np.float32)
        wproj = np.ascontiguousarray(w_proj[256 * g : 256 * (g + 1), :]).astype(
            ml_dtypes.bfloat16
        )
        in_maps.append(
            {"xt": xts[b], "wqkv": np.ascontiguousarray(wqkv), "bqk": bqk,
             "wproj": wproj}
        )
    return in_maps


def kernel(x, w_qkv, b_qkv, w_proj, b_proj):
    x = np.asarray(x, dtype=np.float32)
    w_qkv = np.asarray(w_qkv, dtype=np.float32)
    b_qkv = np.asarray(b_qkv, dtype=np.float32)
    w_proj = np.asarray(w_proj, dtype=np.float32)
    b_proj = np.asarray(b_proj, dtype=np.float32)

    if "nc" not in _CACHE:
        _CACHE["nc"] = build_nc()
    nc = _CACHE["nc"]

    in_maps = _shard_inputs(x, w_qkv, b_qkv, w_proj, b_proj)
    res = run_bass_kernel_spmd(nc, in_maps, list(range(NCORES)))
    # host epilogue: sum head-group partials, add folded bias
    b_eff = (b_qkv[2 * C :].astype(np.float64) @ w_proj.astype(np.float64)
             + b_proj).astype(np.float32)
    out = np.empty((B, T, C), dtype=np.float32)
    for b in range(B):
        acc = res.results[4 * b]["out"].astype(np.float32)
        for g in range(1, 4):
            acc = acc + res.results[4 * b + g]["out"].astype(np.float32)
        out[b] = acc + b_eff
    return out
